# revision 1
# baseline (speedup 1.0000x reference)
"""CombinedGAT (2-layer GAT, N=50000, E=800000) on 8 TRN2 NeuronCores.

Strategy (edge parallelism per sharding hint):
- dst-shard nodes across 8 cores (6250 each); each core owns the edges into
  its shard, sorted by dst, padded to a uniform per-dst-tile chunk count so
  one SPMD program serves all cores.
- Phase A (replicated): h1x table [N, 272] = [h1 (256) | exp(a_src) (8) |
  exp(0.2 a_src) (8)] and adt1 table [N, 16] = [exp(a_dst) | exp(0.2 a_dst)],
  using exp(leakyrelu(u+v)) = max(e^u e^v, e^.2u e^.2v).
- L1 edge pass: per 128-edge chunk, indirect-DMA gather of h1x rows by src;
  attention weights via gathered exps x St-matmul-expanded dst exps; weighted
  scatter-add into per-dst-tile PSUM via one-hot S matmul (S host-built).
- AllGather of compact layer-2 table [6250,10] -> [50000,10]; L2 edge pass
  identical in structure; log_softmax epilogue.
"""
import numpy as np
import ml_dtypes

import concourse.bass as bass
import concourse.mybir as mybir
import concourse.tile as tile
from concourse import bacc
from concourse.bass_utils import run_bass_kernel_spmd

BF = ml_dtypes.bfloat16
P = 128
NCORES = 8
N = 50000
SH = N // NCORES          # 6250 nodes per core
NT = (SH + P - 1) // P    # 49 dst tiles per core
LAST_ROWS = SH - (NT - 1) * P  # 106
HIGH, LOW, EMB = 128, 32, 64
IN1 = HIGH + EMB
HID, HEADS, OUT = 32, 8, 8
IN2 = HID * HEADS
B = 16                    # chunks per super-chunk
NEG = 0.2

AF = mybir.ActivationFunctionType
ALU = mybir.AluOpType


def _prep(inputs):
    """Host-side sharding/layout. Returns per-core in_maps and static dims."""
    ei = np.asarray(inputs["edge_index"])
    src = np.concatenate([ei[0], np.arange(N, dtype=np.int64)])
    dst = np.concatenate([ei[1], np.arange(N, dtype=np.int64)])
    core = dst // SH

    # per-core sorted edge lists
    srcs, dls = [], []
    counts = np.zeros((NCORES, NT), dtype=np.int64)
    for c in range(NCORES):
        m = core == c
        s_c, d_c = src[m], dst[m] - c * SH
        o = np.argsort(d_c, kind="stable")
        s_c, d_c = s_c[o], d_c[o]
        srcs.append(s_c)
        dls.append(d_c)
        counts[c] = np.bincount(d_c // P, minlength=NT)
    C_t = np.maximum(1, np.ceil(counts.max(axis=0) / P).astype(np.int64))  # chunks per tile
    TC = int(C_t.sum())
    NSUP = (TC + B - 1) // B
    tile_of_chunk = np.repeat(np.arange(NT), C_t)
    first_chunk = np.concatenate([[0], np.cumsum(C_t)])[:NT]

    # weight folding
    W1 = np.asarray(inputs["W1"], np.float32)          # [192, 256]
    as1 = np.asarray(inputs["att_src1"], np.float32)   # [8, 32]
    ad1 = np.asarray(inputs["att_dst1"], np.float32)
    W1As = np.einsum("khj,hj->kh", W1.reshape(IN1, HEADS, HID), as1)
    W1Ad = np.einsum("khj,hj->kh", W1.reshape(IN1, HEADS, HID), ad1)
    W1ex = np.concatenate([W1, W1As, W1Ad], axis=1).astype(BF)  # [192, 272]
    W2 = np.asarray(inputs["W2"], np.float32)          # [256, 8]
    W2As = W2 @ np.asarray(inputs["att_src2"], np.float32).reshape(OUT, 1)
    W2Ad = W2 @ np.asarray(inputs["att_dst2"], np.float32).reshape(OUT, 1)
    W2ex = np.concatenate([W2, W2As, W2Ad], axis=1).astype(BF)  # [256, 10]
    Wemb = np.asarray(inputs["W_emb"], np.float32)
    Wemb1 = np.concatenate([Wemb, np.asarray(inputs["b_emb"], np.float32)[None, :]],
                           axis=0).astype(BF)          # [33, 64]

    highT = np.ascontiguousarray(np.asarray(inputs["high_dim_features"], np.float32).T
                                 ).astype(BF)          # [128, N]
    lowT = np.asarray(inputs["low_dim_features"], np.float32).T  # [32, N]
    lowT1 = np.concatenate([lowT, np.ones((1, N), np.float32)], axis=0).astype(BF)

    b1b = np.broadcast_to(np.asarray(inputs["b1"], np.float32), (P, IN2)).copy()
    b2b = np.broadcast_to(np.asarray(inputs["b2"], np.float32), (P, OUT)).copy()
    idn = np.eye(P, dtype=np.float32).astype(BF)

    shared = {
        "highT": highT, "lowT1": lowT1, "W1ex_t": np.ascontiguousarray(W1ex[:HIGH]),
        "W1ex_b": np.ascontiguousarray(W1ex[HIGH:]), "Wemb1": Wemb1,
        "W2ex": np.ascontiguousarray(W2ex.reshape(2, P, 10)),
        "b1b": b1b, "b2b": b2b, "idn": idn,
        "iot": np.broadcast_to(np.arange(P, dtype=np.float32), (P, P)).astype(BF).copy(),
    }

    in_maps = []
    ar = np.arange(P, dtype=np.int16)
    for c in range(NCORES):
        s_c, d_c = srcs[c], dls[c]
        NSLOT = TC * P
        srcg = np.zeros((TC, P), np.int32)
        dl128 = np.full((TC, P), -1, np.int16)
        for t in range(NT):
            sel = (d_c // P) == t
            n_t = int(sel.sum())
            e_s, e_d = s_c[sel], d_c[sel] % P
            base = int(first_chunk[t])
            ch = np.arange(n_t) // P + base
            pp = np.arange(n_t) % P
            srcg[ch, pp] = e_s
            dl128[ch, pp] = e_d
        S = (dl128[:, :, None] == ar[None, None, :]).astype(BF)   # [TC, e, d]
        St = np.ascontiguousarray(S.transpose(0, 2, 1))           # [TC, d, e]
        # pad chunk dim to NSUP*B
        padc = NSUP * B - TC
        if padc:
            srcg = np.concatenate([srcg, np.zeros((padc, P), np.int32)])
            dl128 = np.concatenate([dl128, np.full((padc, P), -1, np.int16)])
            St = np.concatenate([St, np.zeros((padc, P, P), BF)])
        # device layouts
        srcg_dev = np.ascontiguousarray(
            srcg.reshape(NSUP, B, P).transpose(0, 2, 1))          # [NSUP, P, B]
        dl_dev = np.ascontiguousarray(
            dl128.reshape(NSUP, B, P).transpose(0, 2, 1)).astype(BF)  # [NSUP, P, B]
        st_dev = np.ascontiguousarray(
            St.reshape(NSUP, B, P, P).transpose(0, 2, 1, 3).reshape(NSUP, P, B * P))
        dstt = np.minimum(np.arange(NT)[:, None] * P + np.arange(P)[None, :],
                          SH - 1).astype(np.int32) + c * SH       # [NT, P]
        im = dict(shared)
        im.update({"SRCG": srcg_dev, "DL": dl_dev, "DSTT": dstt})
        in_maps.append(im)
    return in_maps, C_t, TC, NSUP, tile_of_chunk, first_chunk


def _build(C_t, TC, NSUP, tile_of_chunk, first_chunk):
    nc = bacc.Bacc("TRN2", target_bir_lowering=False, debug=False, num_devices=NCORES)
    bf, f32, i32 = mybir.dt.bfloat16, mybir.dt.float32, mybir.dt.int32

    highT = nc.dram_tensor("highT", [HIGH, N], bf, kind="ExternalInput")
    lowT1 = nc.dram_tensor("lowT1", [LOW + 1, N], bf, kind="ExternalInput")
    W1ex_t = nc.dram_tensor("W1ex_t", [HIGH, 272], bf, kind="ExternalInput")
    W1ex_b = nc.dram_tensor("W1ex_b", [EMB, 272], bf, kind="ExternalInput")
    Wemb1 = nc.dram_tensor("Wemb1", [LOW + 1, EMB], bf, kind="ExternalInput")
    W2ex = nc.dram_tensor("W2ex", [2, P, 10], bf, kind="ExternalInput")
    b1b = nc.dram_tensor("b1b", [P, IN2], f32, kind="ExternalInput")
    b2b = nc.dram_tensor("b2b", [P, OUT], f32, kind="ExternalInput")
    idn = nc.dram_tensor("idn", [P, P], bf, kind="ExternalInput")
    SRCG = nc.dram_tensor("SRCG", [NSUP, P, B], i32, kind="ExternalInput")
    DL_in = nc.dram_tensor("DL", [NSUP, P, B], bf, kind="ExternalInput")
    iot_in = nc.dram_tensor("iot", [P, P], bf, kind="ExternalInput")
    DSTT = nc.dram_tensor("DSTT", [NT, P], i32, kind="ExternalInput")
    out_d = nc.dram_tensor("out", [SH, OUT], f32, kind="ExternalOutput")

    h1x = nc.dram_tensor("h1x", [N, 272], bf)
    adt1 = nc.dram_tensor("adt1", [N, 16], bf)
    adt2 = nc.dram_tensor("adt2", [SH, 2], bf)

    NTG = N // P + (1 if N % P else 0)  # 391 node tiles (N = 390*128 + 80)

    with tile.TileContext(nc) as tc:
        with tc.tile_pool(name="const", bufs=1) as cpool, \
             tc.tile_pool(name="sb", bufs=3) as sb, \
             tc.tile_pool(name="gat", bufs=3) as gat, \
             tc.tile_pool(name="psA", bufs=2, space="PSUM") as psA, \
             tc.tile_pool(name="psB", bufs=3, space="PSUM") as psB, \
             tc.tile_pool(name="dram", bufs=1, space="DRAM") as dram:

            w1t = cpool.tile([HIGH, 272], bf)
            nc.sync.dma_start(out=w1t[:], in_=W1ex_t[:])
            w1b = cpool.tile([EMB, 272], bf)
            nc.sync.dma_start(out=w1b[:], in_=W1ex_b[:])
            wem = cpool.tile([LOW + 1, EMB], bf)
            nc.sync.dma_start(out=wem[:], in_=Wemb1[:])
            w2e = cpool.tile([P, 2, 10], bf)
            nc.sync.dma_start(out=w2e[:], in_=W2ex[:].rearrange("k p c -> p k c"))
            b1s = cpool.tile([P, IN2], f32)
            nc.sync.dma_start(out=b1s[:], in_=b1b[:])
            b2s = cpool.tile([P, OUT], f32)
            nc.sync.dma_start(out=b2s[:], in_=b2b[:])
            ids = cpool.tile([P, P], bf)
            nc.sync.dma_start(out=ids[:], in_=idn[:])
            iot = cpool.tile([P, P], bf)
            nc.sync.dma_start(out=iot[:], in_=iot_in[:])

            # ---------------- Phase A: tables for all N nodes ----------------
            for ntile in range(NTG):
                n0 = ntile * P
                w = min(P, N - n0)
                ht = sb.tile([P, P], bf, tag="ht")
                nc.sync.dma_start(out=ht[:, :w], in_=highT[:, n0:n0 + w])
                lt = sb.tile([LOW + 1, P], bf, tag="lt")
                nc.sync.dma_start(out=lt[:, :w], in_=lowT1[:, n0:n0 + w])
                embp = psB.tile([EMB, P], f32, tag="pB")
                nc.tensor.matmul(out=embp[:, :w], lhsT=wem[:], rhs=lt[:, :w],
                                 start=True, stop=True)
                # elu(v) = max(v,0)-1 + exp(-relu(-v))
                tm = sb.tile([EMB, P], f32, tag="tm")
                nc.scalar.activation(tm[:, :w], embp[:, :w], AF.Relu, scale=-1.0)
                te = sb.tile([EMB, P], f32, tag="te")
                nc.scalar.activation(te[:, :w], tm[:, :w], AF.Exp, scale=-1.0)
                tr = sb.tile([EMB, P], f32, tag="tr")
                nc.vector.tensor_scalar(tr[:, :w], embp[:, :w], 0.0, -1.0,
                                        ALU.max, ALU.add)
                embs = sb.tile([EMB, P], bf, tag="embs")
                nc.vector.tensor_tensor(embs[:, :w], tr[:, :w], te[:, :w], ALU.add)
                h1p = psA.tile([P, 512], f32, tag="pA")
                nc.tensor.matmul(out=h1p[:w, 0:272], lhsT=ht[:, :w], rhs=w1t[:],
                                 start=True, stop=False)
                nc.tensor.matmul(out=h1p[:w, 0:272], lhsT=embs[:, :w], rhs=w1b[:],
                                 start=False, stop=True)
                h1s = sb.tile([P, 272], bf, tag="h1s")
                nc.scalar.activation(h1s[:w, 0:256], h1p[:w, 0:256], AF.Copy)
                ads = sb.tile([P, 16], bf, tag="ads")
                nc.scalar.activation(h1s[:w, 256:264], h1p[:w, 256:264], AF.Exp)
                nc.scalar.activation(h1s[:w, 264:272], h1p[:w, 256:264], AF.Exp,
                                     scale=NEG)
                nc.scalar.activation(ads[:w, 0:8], h1p[:w, 264:272], AF.Exp)
                nc.scalar.activation(ads[:w, 8:16], h1p[:w, 264:272], AF.Exp,
                                     scale=NEG)
                nc.sync.dma_start(out=h1x[n0:n0 + w, :], in_=h1s[:w])
                nc.sync.dma_start(out=adt1[n0:n0 + w, :], in_=ads[:w])

            # ---------------- L1 edge pass ----------------
            h2xl = dram.tile([SH, 10], bf)
            h2xf = dram.tile([N, 10], bf)

            acc_of_tile = {}
            adt_of_tile = {}

            def l1_epilogue(t):
                rows = P if t < NT - 1 else LAST_ROWS
                acc = acc_of_tile.pop(t)
                rz = sb.tile([P, 8], f32, tag="rz")
                nc.vector.reciprocal(rz[:rows], acc[:rows, 256:264])
                xr = sb.tile([P, IN2], f32, tag="xr")
                nc.vector.tensor_tensor(
                    xr[:rows], acc[:rows, 0:256].rearrange("p (h j) -> p h j", j=HID),
                    rz[:rows, :, None].to_broadcast([rows, 8, HID]), ALU.mult)
                nc.vector.tensor_tensor(xr[:rows], xr[:rows], b1s[:rows], ALU.add)
                tm = sb.tile([P, IN2], f32, tag="etm")
                nc.scalar.activation(tm[:rows], xr[:rows], AF.Relu, scale=-1.0)
                te = sb.tile([P, IN2], f32, tag="ete")
                nc.scalar.activation(te[:rows], tm[:rows], AF.Exp, scale=-1.0)
                tr = sb.tile([P, IN2], f32, tag="etr")
                nc.vector.tensor_scalar(tr[:rows], xr[:rows], 0.0, -1.0,
                                        ALU.max, ALU.add)
                x2 = sb.tile([P, IN2], bf, tag="x2")
                if rows < P:
                    nc.vector.memset(x2[:], 0.0)
                nc.vector.tensor_tensor(x2[:rows], tr[:rows], te[:rows], ALU.add)
                # x2T blocks + h2x row
                x2tb = sb.tile([P, 2, P], bf, tag="x2tb")
                for k in range(2):
                    tp = psB.tile([P, P], bf, tag="pB")
                    nc.tensor.transpose(out=tp[:], in_=x2[:, k * P:(k + 1) * P],
                                        identity=ids[:])
                    nc.vector.tensor_copy(x2tb[:, k, :], tp[:])
                h2p = psB.tile([P, 16], f32, tag="pB")
                for k in range(2):
                    nc.tensor.matmul(out=h2p[:, 0:10], lhsT=x2tb[:, k, :],
                                     rhs=w2e[:, k, :], start=(k == 0), stop=(k == 1))
                h2r = sb.tile([P, 10], bf, tag="h2r")
                nc.scalar.activation(h2r[:rows, 0:8], h2p[:rows, 0:8], AF.Copy)
                nc.scalar.activation(h2r[:rows, 8:9], h2p[:rows, 8:9], AF.Exp)
                nc.scalar.activation(h2r[:rows, 9:10], h2p[:rows, 8:9], AF.Exp,
                                     scale=NEG)
                a2r = sb.tile([P, 2], bf, tag="a2r")
                nc.scalar.activation(a2r[:rows, 0:1], h2p[:rows, 9:10], AF.Exp)
                nc.scalar.activation(a2r[:rows, 1:2], h2p[:rows, 9:10], AF.Exp,
                                     scale=NEG)
                nc.sync.dma_start(out=h2xl[t * P:t * P + rows, :], in_=h2r[:rows])
                nc.sync.dma_start(out=adt2[t * P:t * P + rows, :], in_=a2r[:rows])

            for s in range(NSUP):
                c0 = s * B
                nch = min(B, TC - c0)
                if nch <= 0:
                    break
                it = gat.tile([P, B], i32, tag="it")
                nc.sync.dma_start(out=it[:, :nch], in_=SRCG[s, :, :nch])
                dlt = gat.tile([P, B], bf, tag="dlt")
                nc.sync.dma_start(out=dlt[:, :nch], in_=DL_in[s, :, :nch])
                ssb = gat.tile([P, B * P], bf, tag="ssb")
                nc.vector.tensor_tensor(
                    ssb[:, :nch * P].rearrange("p (b q) -> p b q", q=P),
                    dlt[:, :nch, None].to_broadcast([P, nch, P]),
                    iot[:, None, :].to_broadcast([P, nch, P]), ALU.is_equal)
                sts = gat.tile([P, B * P], bf, tag="sts")
                for ci in range(nch):
                    tpp = psB.tile([P, P], bf, tag="pB", name=f"stp{ci}")
                    nc.tensor.transpose(out=tpp[:], in_=ssb[:, ci * P:(ci + 1) * P],
                                        identity=ids[:])
                    nc.scalar.activation(sts[:, ci * P:(ci + 1) * P], tpp[:], AF.Copy)
                hg = gat.tile([P, B, 272], bf, tag="hg")
                adp = psB.tile([P, B * 16], f32, tag="pAD")
                for ci in range(nch):
                    c = c0 + ci
                    t = int(tile_of_chunk[c])
                    if c == int(first_chunk[t]):
                        dtt = sb.tile([P, 1], i32, tag="dtt")
                        nc.sync.dma_start(out=dtt[:], in_=DSTT[t, :, None])
                        adtt = sb.tile([P, 16], bf, tag=f"adtt{t % 3}")
                        nc.gpsimd.indirect_dma_start(
                            out=adtt[:], out_offset=None, in_=adt1[:],
                            in_offset=bass.IndirectOffsetOnAxis(ap=dtt[:, :1], axis=0))
                        adt_of_tile[t] = adtt
                        acc_of_tile[t] = psA.tile([P, 512], f32, tag="pA", name=f"acc{t}")
                    nc.gpsimd.indirect_dma_start(
                        out=hg[:, ci, :], out_offset=None, in_=h1x[:],
                        in_offset=bass.IndirectOffsetOnAxis(ap=it[:, ci:ci + 1], axis=0))
                    nc.tensor.matmul(out=adp[:, ci * 16:(ci + 1) * 16],
                                     lhsT=sts[:, ci * P:(ci + 1) * P],
                                     rhs=adt_of_tile[t][:], start=True, stop=True)
                # batched attention weights
                t1 = gat.tile([P, B * 8], f32, tag="t1")
                nc.vector.tensor_tensor(
                    t1[:, :nch * 8].rearrange("p (b h) -> p b h", h=8),
                    hg[:, :nch, 256:264],
                    adp[:, :nch * 16].rearrange("p (b h) -> p b h", h=16)[:, :, 0:8],
                    ALU.mult)
                t2 = gat.tile([P, B * 8], f32, tag="t2")
                nc.vector.tensor_tensor(
                    t2[:, :nch * 8].rearrange("p (b h) -> p b h", h=8),
                    hg[:, :nch, 264:272],
                    adp[:, :nch * 16].rearrange("p (b h) -> p b h", h=16)[:, :, 8:16],
                    ALU.mult)
                nc.vector.tensor_tensor(
                    hg[:, :nch, 256:264],
                    t1[:, :nch * 8].rearrange("p (b h) -> p b h", h=8),
                    t2[:, :nch * 8].rearrange("p (b h) -> p b h", h=8),
                    ALU.max)
                nc.vector.tensor_tensor(
                    hg[:, :nch, 0:256].rearrange("p b (h j) -> p b h j", j=HID),
                    hg[:, :nch, 0:256].rearrange("p b (h j) -> p b h j", j=HID),
                    hg[:, :nch, 256:264][:, :, :, None].to_broadcast(
                        [P, nch, 8, HID]),
                    ALU.mult)
                for ci in range(nch):
                    c = c0 + ci
                    t = int(tile_of_chunk[c])
                    last = (c == int(first_chunk[t]) + int(C_t[t]) - 1)
                    nc.tensor.matmul(out=acc_of_tile[t][:, 0:264],
                                     lhsT=ssb[:, ci * P:(ci + 1) * P],
                                     rhs=hg[:, ci, 0:264],
                                     start=(c == int(first_chunk[t])), stop=last)
                    if last:
                        l1_epilogue(t)

            # ---------------- AllGather layer-2 table ----------------
            nc.gpsimd.collective_compute(
                "AllGather", ALU.bypass,
                replica_groups=[list(range(NCORES))],
                ins=[h2xl.opt()], outs=[h2xf.opt()])

            # ---------------- L2 edge pass ----------------
            acc2_of_tile = {}
            adt2_of_tile = {}

            def l2_epilogue(t):
                rows = P if t < NT - 1 else LAST_ROWS
                acc = acc2_of_tile.pop(t)
                rz = sb.tile([P, 1], f32, tag="rz2")
                nc.vector.reciprocal(rz[:rows], acc[:rows, 8:9])
                o = sb.tile([P, OUT], f32, tag="o2")
                nc.vector.tensor_tensor(
                    o[:rows], acc[:rows, 0:8],
                    rz[:rows, :].to_broadcast([rows, OUT]), ALU.mult)
                nc.vector.tensor_tensor(o[:rows], o[:rows], b2s[:rows], ALU.add)
                ex = sb.tile([P, OUT], f32, tag="ex2")
                nc.scalar.activation(ex[:rows], o[:rows], AF.Exp)
                sm = sb.tile([P, 1], f32, tag="sm2")
                nc.vector.reduce_sum(sm[:rows], ex[:rows], axis=mybir.AxisListType.X)
                lg = sb.tile([P, 1], f32, tag="lg2")
                nc.scalar.activation(lg[:rows], sm[:rows], AF.Ln)
                fo = sb.tile([P, OUT], f32, tag="fo2")
                nc.vector.tensor_tensor(
                    fo[:rows], o[:rows],
                    lg[:rows, :].to_broadcast([rows, OUT]), ALU.subtract)
                nc.sync.dma_start(out=out_d[t * P:t * P + rows, :], in_=fo[:rows])

            for s in range(NSUP):
                c0 = s * B
                nch = min(B, TC - c0)
                if nch <= 0:
                    break
                it = gat.tile([P, B], i32, tag="it")
                nc.sync.dma_start(out=it[:, :nch], in_=SRCG[s, :, :nch])
                dlt = gat.tile([P, B], bf, tag="dlt")
                nc.sync.dma_start(out=dlt[:, :nch], in_=DL_in[s, :, :nch])
                ssb = gat.tile([P, B * P], bf, tag="ssb")
                nc.vector.tensor_tensor(
                    ssb[:, :nch * P].rearrange("p (b q) -> p b q", q=P),
                    dlt[:, :nch, None].to_broadcast([P, nch, P]),
                    iot[:, None, :].to_broadcast([P, nch, P]), ALU.is_equal)
                sts = gat.tile([P, B * P], bf, tag="sts")
                for ci in range(nch):
                    tpp = psB.tile([P, P], bf, tag="pB", name=f"stp{ci}")
                    nc.tensor.transpose(out=tpp[:], in_=ssb[:, ci * P:(ci + 1) * P],
                                        identity=ids[:])
                    nc.scalar.activation(sts[:, ci * P:(ci + 1) * P], tpp[:], AF.Copy)
                hg2 = gat.tile([P, B, 10], bf, tag="hg2")
                adp2 = psB.tile([P, B * 2], f32, tag="pAD")
                for ci in range(nch):
                    c = c0 + ci
                    t = int(tile_of_chunk[c])
                    if c == int(first_chunk[t]):
                        a2t = sb.tile([P, 2], bf, tag=f"a2t{t % 3}")
                        rows = P if t < NT - 1 else LAST_ROWS
                        if rows < P:
                            nc.vector.memset(a2t[:], 0.0)
                        nc.sync.dma_start(out=a2t[:rows],
                                          in_=adt2[t * P:t * P + rows, :])
                        adt2_of_tile[t] = a2t
                        acc2_of_tile[t] = psA.tile([P, 512], f32, tag="pA", name=f"acc2_{t}")
                    nc.gpsimd.indirect_dma_start(
                        out=hg2[:, ci, :], out_offset=None, in_=h2xf[:],
                        in_offset=bass.IndirectOffsetOnAxis(ap=it[:, ci:ci + 1], axis=0))
                    nc.tensor.matmul(out=adp2[:, ci * 2:(ci + 1) * 2],
                                     lhsT=sts[:, ci * P:(ci + 1) * P],
                                     rhs=adt2_of_tile[t][:], start=True, stop=True)
                t1 = gat.tile([P, B], f32, tag="t1b")
                nc.vector.tensor_tensor(
                    t1[:, :nch, None], hg2[:, :nch, 8:9],
                    adp2[:, :nch * 2].rearrange("p (b k) -> p b k", k=2)[:, :, 0:1],
                    ALU.mult)
                t2 = gat.tile([P, B], f32, tag="t2b")
                nc.vector.tensor_tensor(
                    t2[:, :nch, None], hg2[:, :nch, 9:10],
                    adp2[:, :nch * 2].rearrange("p (b k) -> p b k", k=2)[:, :, 1:2],
                    ALU.mult)
                nc.vector.tensor_tensor(
                    hg2[:, :nch, 8:9], t1[:, :nch, None], t2[:, :nch, None], ALU.max)
                nc.vector.tensor_tensor(
                    hg2[:, :nch, 0:8], hg2[:, :nch, 0:8],
                    hg2[:, :nch, 8:9].to_broadcast([P, nch, OUT]), ALU.mult)
                for ci in range(nch):
                    c = c0 + ci
                    t = int(tile_of_chunk[c])
                    last = (c == int(first_chunk[t]) + int(C_t[t]) - 1)
                    nc.tensor.matmul(out=acc2_of_tile[t][:, 0:9],
                                     lhsT=ssb[:, ci * P:(ci + 1) * P],
                                     rhs=hg2[:, ci, 0:9],
                                     start=(c == int(first_chunk[t])), stop=last)
                    if last:
                        l2_epilogue(t)

    if not nc.is_finalized():
        nc.finalize()
    return nc


_CACHE = {}


def kernel(**inputs):
    in_maps, C_t, TC, NSUP, tile_of_chunk, first_chunk = _prep(inputs)
    key = (TC, NSUP)
    if key not in _CACHE:
        _CACHE[key] = _build(C_t, TC, NSUP, tile_of_chunk, first_chunk)
    nc = _CACHE[key]
    res = run_bass_kernel_spmd(nc, in_maps, list(range(NCORES)))
    out = np.empty((N, OUT), np.float32)
    for c in range(NCORES):
        out[c * SH:(c + 1) * SH] = res.results[c]["out"]
    return out



# revision 4
# speedup vs baseline: 8.9946x; 8.9946x over previous
"""CombinedGAT (2-layer GAT, N=50000, E=800000) on 8 TRN2 NeuronCores.

Strategy (edge parallelism per sharding hint):
- dst-shard nodes across 8 cores (6250 each); each core owns the edges into
  its shard, sorted by dst, padded to a uniform per-dst-tile chunk count so
  one SPMD program serves all cores.
- Phase A is *node-sharded*: core c computes the layer-1 table rows for its
  own 6250 nodes only ([SH, 272] = [h1 (256) | exp(a_src) (8) |
  exp(0.2 a_src) (8)]) using exp(leakyrelu(u+v)) = max(e^u e^v, e^.2u e^.2v),
  then an AllGather replicates the full [N, 272] table. The dst-side exp
  table adt1 [SH, 16] stays local (dst always lands in the owner's shard).
- L1 edge pass: per 128-edge chunk, indirect-DMA gather of h1x rows by src;
  attention weights via gathered exps x St-matmul-expanded dst exps; weighted
  scatter-add into per-dst-tile PSUM via one-hot S matmul (S built on device
  from compact dst-local bytes).
- AllGather of compact layer-2 table [6250,10] -> [50000,10]; L2 edge pass
  identical in structure; log_softmax epilogue.

I/O strategy (the axon tunnel is ~84 MB/s with ~10ms per shard transfer, so
bytes and array count dominate wall time): features are uploaded *sharded*
(2 MB/core instead of 16 MB/core replicated) as one bf16 array, and all
remaining per-core data (edge chunk tables, weights, biases, iota/identity
constants) is packed into ONE int32 blob per core, with bf16 sections read
on device via bitcast APs. Uploads are issued asynchronously so the feature
transfer overlaps the host-side edge bucketing.
"""
import numpy as np
import ml_dtypes

import jax
from jax.sharding import Mesh, NamedSharding, PartitionSpec
from jax.experimental.shard_map import shard_map

import concourse.bass as bass
import concourse.mybir as mybir
import concourse.tile as tile
from concourse import bacc

BF = ml_dtypes.bfloat16
P = 128
NCORES = 8
N = 50000
SH = N // NCORES          # 6250 nodes per core
NT = (SH + P - 1) // P    # 49 dst tiles per core
LAST_ROWS = SH - (NT - 1) * P  # 106
HIGH, LOW, EMB = 128, 32, 64
IN1 = HIGH + EMB
HID, HEADS, OUT = 32, 8, 8
IN2 = HID * HEADS
B = 16                    # chunks per super-chunk
NEG = 0.2
FROWS = HIGH + LOW + 1    # feature blob rows per core: highT | lowT | ones

AF = mybir.ActivationFunctionType
ALU = mybir.AluOpType


# ---------------------------------------------------------------- blob layout
def _blob_layout(NSUP):
    """int32 blob: [SRCG i32 | DSTT i32 | bf16 sections (bitcast)]."""
    S1 = NSUP * P * B
    S2 = NT * P
    secs = {}
    off = S1 + S2
    for name, shape in [
        ("DL", (NSUP, P, B)),
        ("W1t", (HIGH, 272)),
        ("W1b", (EMB, 272)),
        ("Wemb", (LOW + 1, EMB)),
        ("W2e", (P, 2, 10)),
        ("idn", (P, P)),
        ("iot", (P, P)),
        ("b1b", (P, IN2)),
        ("b2b", (P, OUT)),
    ]:
        n = int(np.prod(shape))
        assert n % 2 == 0
        secs[name] = (off, n, shape)
        off += n // 2
    return S1, S2, secs, off


# ---------------------------------------------------------------- host prep
def _prep_feat(inputs):
    """[8*FROWS, SH] bf16: per core rows = [high^T (128) | low^T (32) | ones]."""
    high = np.asarray(inputs["high_dim_features"], np.float32)
    low = np.asarray(inputs["low_dim_features"], np.float32)
    FG = np.empty((NCORES, FROWS, SH), BF)
    FG[:, :HIGH, :] = high.reshape(NCORES, SH, HIGH).transpose(0, 2, 1)
    FG[:, HIGH:HIGH + LOW, :] = low.reshape(NCORES, SH, LOW).transpose(0, 2, 1)
    FG[:, HIGH + LOW, :] = np.float32(1.0)
    return FG.reshape(NCORES * FROWS, SH)


def _prep_edges(inputs):
    """Bucket edges by (dst core, dst tile) into 128-edge chunks; pack blob."""
    ei = np.asarray(inputs["edge_index"])
    loops = np.arange(N, dtype=np.int32)
    src = np.concatenate([ei[0].astype(np.int32), loops])
    dst = np.concatenate([ei[1].astype(np.int32), loops])
    o = np.argsort(dst, kind="stable")
    ss, ds = src[o], dst[o]
    core = ds // SH
    dl = ds - core * SH
    tg = core * NT + dl // P                      # global tile id, ascending
    starts = np.searchsorted(tg, np.arange(NCORES * NT + 1))
    cnt = np.diff(starts).reshape(NCORES, NT)
    C_t = np.maximum(1, np.ceil(cnt.max(axis=0) / P).astype(np.int64))
    TC = int(C_t.sum())
    NSUP = (TC + B - 1) // B
    # pad the last tile's chunk range to the full NSUP*B slot count so the
    # device loop is uniform (pad chunks have dl=-1 -> zero one-hot)
    C_t[NT - 1] += NSUP * B - TC
    TC = NSUP * B
    first_chunk = np.concatenate([[0], np.cumsum(C_t)])[:NT]
    tile_of_chunk = np.repeat(np.arange(NT), C_t)

    pos = np.arange(len(ds)) - starts[tg]
    tloc = tg - core * NT
    flat = (core.astype(np.int64) * TC + first_chunk[tloc] + pos // P) * P \
        + pos % P
    srcg = np.zeros((NCORES, TC, P), np.int32)
    dlc = np.full((NCORES, TC, P), -1, np.int16)
    srcg.reshape(-1)[flat] = ss
    dlc.reshape(-1)[flat] = (dl % P).astype(np.int16)
    srcg_dev = np.ascontiguousarray(
        srcg.reshape(NCORES, NSUP, B, P).transpose(0, 1, 3, 2))  # [8,NSUP,P,B]
    dl_dev = np.ascontiguousarray(
        dlc.reshape(NCORES, NSUP, B, P).transpose(0, 1, 3, 2)).astype(BF)

    S1, S2, secs, Lr = _blob_layout(NSUP)
    blob = np.empty((NCORES, Lr), np.int32)
    blob[:, :S1] = srcg_dev.reshape(NCORES, S1)
    dstt = np.minimum(np.arange(NT)[:, None] * P + np.arange(P)[None, :],
                      SH - 1).astype(np.int32)                   # local ids
    blob[:, S1:S1 + S2] = dstt.reshape(-1)[None, :]
    off, n, _ = secs["DL"]
    blob[:, off:off + n // 2] = dl_dev.reshape(NCORES, n).view(np.int32)

    # weight folding (replicated across cores)
    W1 = np.asarray(inputs["W1"], np.float32)          # [192, 256]
    as1 = np.asarray(inputs["att_src1"], np.float32)
    ad1 = np.asarray(inputs["att_dst1"], np.float32)
    W1As = np.einsum("khj,hj->kh", W1.reshape(IN1, HEADS, HID), as1)
    W1Ad = np.einsum("khj,hj->kh", W1.reshape(IN1, HEADS, HID), ad1)
    W1ex = np.concatenate([W1, W1As, W1Ad], axis=1).astype(BF)   # [192, 272]
    W2 = np.asarray(inputs["W2"], np.float32)          # [256, 8]
    W2As = W2 @ np.asarray(inputs["att_src2"], np.float32).reshape(OUT, 1)
    W2Ad = W2 @ np.asarray(inputs["att_dst2"], np.float32).reshape(OUT, 1)
    W2ex = np.concatenate([W2, W2As, W2Ad], axis=1).astype(BF)   # [256, 10]
    Wemb = np.asarray(inputs["W_emb"], np.float32)
    Wemb1 = np.concatenate(
        [Wemb, np.asarray(inputs["b_emb"], np.float32)[None, :]],
        axis=0).astype(BF)                             # [33, 64]
    wparts = {
        "W1t": np.ascontiguousarray(W1ex[:HIGH]),
        "W1b": np.ascontiguousarray(W1ex[HIGH:]),
        "Wemb": Wemb1,
        "W2e": np.ascontiguousarray(
            W2ex.reshape(2, P, 10).transpose(1, 0, 2)),          # [P, 2, 10]
        "idn": np.eye(P, dtype=np.float32).astype(BF),
        "iot": np.broadcast_to(np.arange(P, dtype=np.float32),
                               (P, P)).astype(BF).copy(),
        "b1b": np.broadcast_to(np.asarray(inputs["b1"], np.float32),
                               (P, IN2)).astype(BF).copy(),
        "b2b": np.broadcast_to(np.asarray(inputs["b2"], np.float32),
                               (P, OUT)).astype(BF).copy(),
    }
    for name, arr in wparts.items():
        off, n, shape = secs[name]
        assert arr.shape == shape, (name, arr.shape, shape)
        blob[:, off:off + n // 2] = arr.reshape(-1).view(np.int32)[None, :]

    return blob, C_t, TC, NSUP, tile_of_chunk, first_chunk, Lr


# ---------------------------------------------------------------- device build
def _build(C_t, TC, NSUP, tile_of_chunk, first_chunk, Lr):
    nc = bacc.Bacc("TRN2", target_bir_lowering=False, debug=False,
                   num_devices=NCORES)
    bf, f32, i32 = mybir.dt.bfloat16, mybir.dt.float32, mybir.dt.int32

    FEAT = nc.dram_tensor("FEAT", [FROWS, SH], bf, kind="ExternalInput")
    BLOB = nc.dram_tensor("BLOB", [Lr], i32, kind="ExternalInput")
    out_d = nc.dram_tensor("out", [SH, OUT], f32, kind="ExternalOutput")

    adt1 = nc.dram_tensor("adt1", [SH, 16], bf)
    adt2 = nc.dram_tensor("adt2", [SH, 2], bf)

    S1, S2, secs, Lr2 = _blob_layout(NSUP)
    assert Lr2 == Lr

    def sec_ap(name):
        off, n, shape = secs[name]
        ap = BLOB[off:off + n // 2].bitcast(bf)
        if len(shape) == 2:
            return ap.rearrange("(a b) -> a b", b=shape[1])
        return ap.rearrange("(a b c) -> a b c", b=shape[1], c=shape[2])

    def srcg_ap(s):
        return BLOB[s * P * B:(s + 1) * P * B].rearrange("(p b) -> p b", b=B)

    def dl_ap(s):
        off = secs["DL"][0]
        return BLOB[off + s * P * B // 2:off + (s + 1) * P * B // 2] \
            .bitcast(bf).rearrange("(p b) -> p b", b=B)

    def dstt_ap(t):
        return BLOB[S1 + t * P:S1 + (t + 1) * P].rearrange("(p a) -> p a", a=1)

    with tile.TileContext(nc) as tc:
        with tc.tile_pool(name="const", bufs=1) as cpool, \
             tc.tile_pool(name="sb", bufs=3) as sb, \
             tc.tile_pool(name="gat", bufs=3) as gat, \
             tc.tile_pool(name="psA", bufs=2, space="PSUM") as psA, \
             tc.tile_pool(name="psB", bufs=3, space="PSUM") as psB, \
             tc.tile_pool(name="dram", bufs=1, space="DRAM") as dram:

            h1l = dram.tile([SH, 272], bf)
            h1x = dram.tile([N, 272], bf)

            w1t = cpool.tile([HIGH, 272], bf)
            nc.sync.dma_start(out=w1t[:], in_=sec_ap("W1t"))
            w1b = cpool.tile([EMB, 272], bf)
            nc.sync.dma_start(out=w1b[:], in_=sec_ap("W1b"))
            wem = cpool.tile([LOW + 1, EMB], bf)
            nc.sync.dma_start(out=wem[:], in_=sec_ap("Wemb"))
            w2e = cpool.tile([P, 2, 10], bf)
            nc.sync.dma_start(out=w2e[:], in_=sec_ap("W2e"))
            b1s = cpool.tile([P, IN2], bf)
            nc.sync.dma_start(out=b1s[:], in_=sec_ap("b1b"))
            b2s = cpool.tile([P, OUT], bf)
            nc.sync.dma_start(out=b2s[:], in_=sec_ap("b2b"))
            ids = cpool.tile([P, P], bf)
            nc.sync.dma_start(out=ids[:], in_=sec_ap("idn"))
            iot = cpool.tile([P, P], bf)
            nc.sync.dma_start(out=iot[:], in_=sec_ap("iot"))

            # -------- Phase A: layer-1 tables for this core's SH nodes -------
            for ntile in range(NT):
                n0 = ntile * P
                w = min(P, SH - n0)
                ht = sb.tile([P, P], bf, tag="ht")
                nc.sync.dma_start(out=ht[:, :w], in_=FEAT[0:HIGH, n0:n0 + w])
                lt = sb.tile([LOW + 1, P], bf, tag="lt")
                nc.sync.dma_start(out=lt[:, :w], in_=FEAT[HIGH:FROWS, n0:n0 + w])
                embp = psB.tile([EMB, P], f32, tag="pB")
                nc.tensor.matmul(out=embp[:, :w], lhsT=wem[:], rhs=lt[:, :w],
                                 start=True, stop=True)
                # elu(v) = max(v,0)-1 + exp(-relu(-v))
                tm = sb.tile([EMB, P], f32, tag="tm")
                nc.scalar.activation(tm[:, :w], embp[:, :w], AF.Relu, scale=-1.0)
                te = sb.tile([EMB, P], f32, tag="te")
                nc.scalar.activation(te[:, :w], tm[:, :w], AF.Exp, scale=-1.0)
                tr = sb.tile([EMB, P], f32, tag="tr")
                nc.vector.tensor_scalar(tr[:, :w], embp[:, :w], 0.0, -1.0,
                                        ALU.max, ALU.add)
                embs = sb.tile([EMB, P], bf, tag="embs")
                nc.vector.tensor_tensor(embs[:, :w], tr[:, :w], te[:, :w],
                                        ALU.add)
                h1p = psA.tile([P, 512], f32, tag="pA")
                nc.tensor.matmul(out=h1p[:w, 0:272], lhsT=ht[:, :w], rhs=w1t[:],
                                 start=True, stop=False)
                nc.tensor.matmul(out=h1p[:w, 0:272], lhsT=embs[:, :w],
                                 rhs=w1b[:], start=False, stop=True)
                h1s = sb.tile([P, 272], bf, tag="h1s")
                nc.scalar.activation(h1s[:w, 0:256], h1p[:w, 0:256], AF.Copy)
                ads = sb.tile([P, 16], bf, tag="ads")
                nc.scalar.activation(h1s[:w, 256:264], h1p[:w, 256:264], AF.Exp)
                nc.scalar.activation(h1s[:w, 264:272], h1p[:w, 256:264], AF.Exp,
                                     scale=NEG)
                nc.scalar.activation(ads[:w, 0:8], h1p[:w, 264:272], AF.Exp)
                nc.scalar.activation(ads[:w, 8:16], h1p[:w, 264:272], AF.Exp,
                                     scale=NEG)
                nc.sync.dma_start(out=h1l[n0:n0 + w, :], in_=h1s[:w])
                nc.sync.dma_start(out=adt1[n0:n0 + w, :], in_=ads[:w])

            # -------- replicate the layer-1 table --------
            nc.gpsimd.collective_compute(
                "AllGather", ALU.bypass,
                replica_groups=[list(range(NCORES))],
                ins=[h1l.opt()], outs=[h1x.opt()])

            # ---------------- L1 edge pass ----------------
            h2xl = dram.tile([SH, 10], bf)
            h2xf = dram.tile([N, 10], bf)

            acc_of_tile = {}
            adt_of_tile = {}

            def l1_epilogue(t):
                rows = P if t < NT - 1 else LAST_ROWS
                acc = acc_of_tile.pop(t)
                rz = sb.tile([P, 8], f32, tag="rz")
                nc.vector.reciprocal(rz[:rows], acc[:rows, 256:264])
                xr = sb.tile([P, IN2], f32, tag="xr")
                nc.vector.tensor_tensor(
                    xr[:rows],
                    acc[:rows, 0:256].rearrange("p (h j) -> p h j", j=HID),
                    rz[:rows, :, None].to_broadcast([rows, 8, HID]), ALU.mult)
                nc.vector.tensor_tensor(xr[:rows], xr[:rows], b1s[:rows],
                                        ALU.add)
                tm = sb.tile([P, IN2], f32, tag="etm")
                nc.scalar.activation(tm[:rows], xr[:rows], AF.Relu, scale=-1.0)
                te = sb.tile([P, IN2], f32, tag="ete")
                nc.scalar.activation(te[:rows], tm[:rows], AF.Exp, scale=-1.0)
                tr = sb.tile([P, IN2], f32, tag="etr")
                nc.vector.tensor_scalar(tr[:rows], xr[:rows], 0.0, -1.0,
                                        ALU.max, ALU.add)
                x2 = sb.tile([P, IN2], bf, tag="x2")
                if rows < P:
                    nc.vector.memset(x2[:], 0.0)
                nc.vector.tensor_tensor(x2[:rows], tr[:rows], te[:rows],
                                        ALU.add)
                # x2T blocks + h2x row
                x2tb = sb.tile([P, 2, P], bf, tag="x2tb")
                for k in range(2):
                    tp = psB.tile([P, P], bf, tag="pB")
                    nc.tensor.transpose(out=tp[:], in_=x2[:, k * P:(k + 1) * P],
                                        identity=ids[:])
                    nc.vector.tensor_copy(x2tb[:, k, :], tp[:])
                h2p = psB.tile([P, 16], f32, tag="pB")
                for k in range(2):
                    nc.tensor.matmul(out=h2p[:, 0:10], lhsT=x2tb[:, k, :],
                                     rhs=w2e[:, k, :], start=(k == 0),
                                     stop=(k == 1))
                h2r = sb.tile([P, 10], bf, tag="h2r")
                nc.scalar.activation(h2r[:rows, 0:8], h2p[:rows, 0:8], AF.Copy)
                nc.scalar.activation(h2r[:rows, 8:9], h2p[:rows, 8:9], AF.Exp)
                nc.scalar.activation(h2r[:rows, 9:10], h2p[:rows, 8:9], AF.Exp,
                                     scale=NEG)
                a2r = sb.tile([P, 2], bf, tag="a2r")
                nc.scalar.activation(a2r[:rows, 0:1], h2p[:rows, 9:10], AF.Exp)
                nc.scalar.activation(a2r[:rows, 1:2], h2p[:rows, 9:10], AF.Exp,
                                     scale=NEG)
                nc.sync.dma_start(out=h2xl[t * P:t * P + rows, :], in_=h2r[:rows])
                nc.sync.dma_start(out=adt2[t * P:t * P + rows, :], in_=a2r[:rows])

            for s in range(NSUP):
                c0 = s * B
                it = gat.tile([P, B], i32, tag="it")
                nc.sync.dma_start(out=it[:], in_=srcg_ap(s))
                dlt = gat.tile([P, B], bf, tag="dlt")
                nc.sync.dma_start(out=dlt[:], in_=dl_ap(s))
                ssb = gat.tile([P, B * P], bf, tag="ssb")
                nc.vector.tensor_tensor(
                    ssb[:].rearrange("p (b q) -> p b q", q=P),
                    dlt[:, :, None].to_broadcast([P, B, P]),
                    iot[:, None, :].to_broadcast([P, B, P]), ALU.is_equal)
                sts = gat.tile([P, B * P], bf, tag="sts")
                for ci in range(B):
                    tpp = psB.tile([P, P], bf, tag="pB", name=f"stp{ci}")
                    nc.tensor.transpose(out=tpp[:],
                                        in_=ssb[:, ci * P:(ci + 1) * P],
                                        identity=ids[:])
                    nc.scalar.activation(sts[:, ci * P:(ci + 1) * P], tpp[:],
                                         AF.Copy)
                hg = gat.tile([P, B, 272], bf, tag="hg")
                adp = psB.tile([P, B * 16], f32, tag="pAD")
                for ci in range(B):
                    c = c0 + ci
                    t = int(tile_of_chunk[c])
                    if c == int(first_chunk[t]):
                        dtt = sb.tile([P, 1], i32, tag="dtt")
                        nc.sync.dma_start(out=dtt[:], in_=dstt_ap(t))
                        adtt = sb.tile([P, 16], bf, tag=f"adtt{t % 3}")
                        nc.gpsimd.indirect_dma_start(
                            out=adtt[:], out_offset=None, in_=adt1[:],
                            in_offset=bass.IndirectOffsetOnAxis(
                                ap=dtt[:, :1], axis=0))
                        adt_of_tile[t] = adtt
                        acc_of_tile[t] = psA.tile([P, 512], f32, tag="pA",
                                                  name=f"acc{t}")
                    nc.gpsimd.indirect_dma_start(
                        out=hg[:, ci, :], out_offset=None, in_=h1x[:],
                        in_offset=bass.IndirectOffsetOnAxis(
                            ap=it[:, ci:ci + 1], axis=0))
                    nc.tensor.matmul(out=adp[:, ci * 16:(ci + 1) * 16],
                                     lhsT=sts[:, ci * P:(ci + 1) * P],
                                     rhs=adt_of_tile[t][:], start=True,
                                     stop=True)
                # batched attention weights
                t1 = gat.tile([P, B * 8], f32, tag="t1")
                nc.vector.tensor_tensor(
                    t1[:].rearrange("p (b h) -> p b h", h=8),
                    hg[:, :, 256:264],
                    adp[:].rearrange("p (b h) -> p b h", h=16)[:, :, 0:8],
                    ALU.mult)
                t2 = gat.tile([P, B * 8], f32, tag="t2")
                nc.vector.tensor_tensor(
                    t2[:].rearrange("p (b h) -> p b h", h=8),
                    hg[:, :, 264:272],
                    adp[:].rearrange("p (b h) -> p b h", h=16)[:, :, 8:16],
                    ALU.mult)
                nc.vector.tensor_tensor(
                    hg[:, :, 256:264],
                    t1[:].rearrange("p (b h) -> p b h", h=8),
                    t2[:].rearrange("p (b h) -> p b h", h=8),
                    ALU.max)
                nc.vector.tensor_tensor(
                    hg[:, :, 0:256].rearrange("p b (h j) -> p b h j", j=HID),
                    hg[:, :, 0:256].rearrange("p b (h j) -> p b h j", j=HID),
                    hg[:, :, 256:264][:, :, :, None].to_broadcast(
                        [P, B, 8, HID]),
                    ALU.mult)
                for ci in range(B):
                    c = c0 + ci
                    t = int(tile_of_chunk[c])
                    last = (c == int(first_chunk[t]) + int(C_t[t]) - 1)
                    nc.tensor.matmul(out=acc_of_tile[t][:, 0:264],
                                     lhsT=ssb[:, ci * P:(ci + 1) * P],
                                     rhs=hg[:, ci, 0:264],
                                     start=(c == int(first_chunk[t])),
                                     stop=last)
                    if last:
                        l1_epilogue(t)

            # ---------------- AllGather layer-2 table ----------------
            nc.gpsimd.collective_compute(
                "AllGather", ALU.bypass,
                replica_groups=[list(range(NCORES))],
                ins=[h2xl.opt()], outs=[h2xf.opt()])

            # ---------------- L2 edge pass ----------------
            acc2_of_tile = {}
            adt2_of_tile = {}

            def l2_epilogue(t):
                rows = P if t < NT - 1 else LAST_ROWS
                acc = acc2_of_tile.pop(t)
                rz = sb.tile([P, 1], f32, tag="rz2")
                nc.vector.reciprocal(rz[:rows], acc[:rows, 8:9])
                o = sb.tile([P, OUT], f32, tag="o2")
                nc.vector.tensor_tensor(
                    o[:rows], acc[:rows, 0:8],
                    rz[:rows, :].to_broadcast([rows, OUT]), ALU.mult)
                nc.vector.tensor_tensor(o[:rows], o[:rows], b2s[:rows], ALU.add)
                ex = sb.tile([P, OUT], f32, tag="ex2")
                nc.scalar.activation(ex[:rows], o[:rows], AF.Exp)
                sm = sb.tile([P, 1], f32, tag="sm2")
                nc.vector.reduce_sum(sm[:rows], ex[:rows],
                                     axis=mybir.AxisListType.X)
                lg = sb.tile([P, 1], f32, tag="lg2")
                nc.scalar.activation(lg[:rows], sm[:rows], AF.Ln)
                fo = sb.tile([P, OUT], f32, tag="fo2")
                nc.vector.tensor_tensor(
                    fo[:rows], o[:rows],
                    lg[:rows, :].to_broadcast([rows, OUT]), ALU.subtract)
                nc.sync.dma_start(out=out_d[t * P:t * P + rows, :], in_=fo[:rows])

            for s in range(NSUP):
                c0 = s * B
                it = gat.tile([P, B], i32, tag="it")
                nc.sync.dma_start(out=it[:], in_=srcg_ap(s))
                dlt = gat.tile([P, B], bf, tag="dlt")
                nc.sync.dma_start(out=dlt[:], in_=dl_ap(s))
                ssb = gat.tile([P, B * P], bf, tag="ssb")
                nc.vector.tensor_tensor(
                    ssb[:].rearrange("p (b q) -> p b q", q=P),
                    dlt[:, :, None].to_broadcast([P, B, P]),
                    iot[:, None, :].to_broadcast([P, B, P]), ALU.is_equal)
                sts = gat.tile([P, B * P], bf, tag="sts")
                for ci in range(B):
                    tpp = psB.tile([P, P], bf, tag="pB", name=f"stp{ci}")
                    nc.tensor.transpose(out=tpp[:],
                                        in_=ssb[:, ci * P:(ci + 1) * P],
                                        identity=ids[:])
                    nc.scalar.activation(sts[:, ci * P:(ci + 1) * P], tpp[:],
                                         AF.Copy)
                hg2 = gat.tile([P, B, 10], bf, tag="hg2")
                adp2 = psB.tile([P, B * 2], f32, tag="pAD")
                for ci in range(B):
                    c = c0 + ci
                    t = int(tile_of_chunk[c])
                    if c == int(first_chunk[t]):
                        a2t = sb.tile([P, 2], bf, tag=f"a2t{t % 3}")
                        rows = P if t < NT - 1 else LAST_ROWS
                        if rows < P:
                            nc.vector.memset(a2t[:], 0.0)
                        nc.sync.dma_start(out=a2t[:rows],
                                          in_=adt2[t * P:t * P + rows, :])
                        adt2_of_tile[t] = a2t
                        acc2_of_tile[t] = psA.tile([P, 512], f32, tag="pA",
                                                   name=f"acc2_{t}")
                    nc.gpsimd.indirect_dma_start(
                        out=hg2[:, ci, :], out_offset=None, in_=h2xf[:],
                        in_offset=bass.IndirectOffsetOnAxis(
                            ap=it[:, ci:ci + 1], axis=0))
                    nc.tensor.matmul(out=adp2[:, ci * 2:(ci + 1) * 2],
                                     lhsT=sts[:, ci * P:(ci + 1) * P],
                                     rhs=adt2_of_tile[t][:], start=True,
                                     stop=True)
                t1 = gat.tile([P, B], f32, tag="t1b")
                nc.vector.tensor_tensor(
                    t1[:, :, None], hg2[:, :, 8:9],
                    adp2[:].rearrange("p (b k) -> p b k", k=2)[:, :, 0:1],
                    ALU.mult)
                t2 = gat.tile([P, B], f32, tag="t2b")
                nc.vector.tensor_tensor(
                    t2[:, :, None], hg2[:, :, 9:10],
                    adp2[:].rearrange("p (b k) -> p b k", k=2)[:, :, 1:2],
                    ALU.mult)
                nc.vector.tensor_tensor(
                    hg2[:, :, 8:9], t1[:, :, None], t2[:, :, None], ALU.max)
                nc.vector.tensor_tensor(
                    hg2[:, :, 0:8], hg2[:, :, 0:8],
                    hg2[:, :, 8:9].to_broadcast([P, B, OUT]), ALU.mult)
                for ci in range(B):
                    c = c0 + ci
                    t = int(tile_of_chunk[c])
                    last = (c == int(first_chunk[t]) + int(C_t[t]) - 1)
                    nc.tensor.matmul(out=acc2_of_tile[t][:, 0:9],
                                     lhsT=ssb[:, ci * P:(ci + 1) * P],
                                     rhs=hg2[:, ci, 0:9],
                                     start=(c == int(first_chunk[t])),
                                     stop=last)
                    if last:
                        l2_epilogue(t)

    if not nc.is_finalized():
        nc.finalize()
    return nc


# ---------------------------------------------------------------- runner
_CACHE = {}   # structure key -> (nc, runner)


def _make_runner(nc):
    """Cached-jit replica of bass2jax.run_bass_via_pjrt (axon path)."""
    from concourse.bass2jax import (install_neuronx_cc_hook,
                                    partition_id_tensor, _bass_exec_p)
    install_neuronx_cc_hook()
    partition_name = (nc.partition_id_tensor.name
                      if nc.partition_id_tensor else None)
    in_names, out_names, out_avals = [], [], []
    for alloc in nc.m.functions[0].allocations:
        if not isinstance(alloc, mybir.MemoryLocationSet):
            continue
        name = alloc.memorylocations[0].name
        if alloc.kind == "ExternalInput":
            if name != partition_name:
                in_names.append(name)
        elif alloc.kind == "ExternalOutput":
            out_names.append(name)
            out_avals.append(jax.core.ShapedArray(
                tuple(alloc.tensor_shape), mybir.dt.np(alloc.dtype)))
    n_params = len(in_names)
    n_outs = len(out_avals)
    in_names_all = in_names + out_names + (
        [partition_name] if partition_name else [])
    donate = tuple(range(n_params, n_params + n_outs))

    def _body(*args):
        operands = list(args)
        if partition_name is not None:
            operands.append(partition_id_tensor())
        outs = _bass_exec_p.bind(
            *operands, out_avals=tuple(out_avals),
            in_names=tuple(in_names_all), out_names=tuple(out_names),
            lowering_input_output_aliases=(), sim_require_finite=True,
            sim_require_nnan=True, nc=nc)
        return tuple(outs)

    devices = jax.devices()[:NCORES]
    mesh = Mesh(np.asarray(devices), ("core",))
    sharding = NamedSharding(mesh, PartitionSpec("core"))
    in_specs = (PartitionSpec("core"),) * (n_params + n_outs)
    out_specs = (PartitionSpec("core"),) * len(out_names)
    fn = jax.jit(
        shard_map(_body, mesh=mesh, in_specs=in_specs, out_specs=out_specs,
                  check_rep=False),
        donate_argnums=donate, keep_unused=True)
    return fn, in_names, out_names, out_avals, sharding


def kernel(**inputs):
    FG = _prep_feat(inputs)
    # fire the (dominant) feature upload before doing edge bucketing so the
    # tunnel transfer overlaps host prep
    dfeat = dzeros = None
    try:
        devices = jax.devices()[:NCORES]
        mesh = Mesh(np.asarray(devices), ("core",))
        sharding = NamedSharding(mesh, PartitionSpec("core"))
        dfeat = jax.device_put(FG, sharding)
        dzeros = jax.device_put(
            np.zeros((NCORES * SH, OUT), np.float32), sharding)
    except Exception:
        dfeat = dzeros = None

    blob, C_t, TC, NSUP, toc, fc, Lr = _prep_edges(inputs)
    key = (TC, NSUP, tuple(int(x) for x in C_t))
    if key not in _CACHE:
        nc = _build(C_t, TC, NSUP, toc, fc, Lr)
        _CACHE[key] = (nc, _make_runner(nc))
    nc, (fn, in_names, out_names, out_avals, sharding) = _CACHE[key]

    host_in = {"FEAT": FG, "BLOB": blob.reshape(-1)}
    dev_in = []
    for name in in_names:
        if name == "FEAT" and dfeat is not None:
            dev_in.append(dfeat)
        else:
            dev_in.append(jax.device_put(host_in[name], sharding))
    if dzeros is None:
        dzeros = jax.device_put(
            np.zeros((NCORES * SH, OUT), np.float32), sharding)
    out_arrs = fn(*dev_in, dzeros)
    out = np.asarray(out_arrs[out_names.index("out")])
    return out.reshape(N, OUT)


# revision 7
# speedup vs baseline: 12.2251x; 1.3592x over previous
"""CombinedGAT (2-layer GAT, N=50000, E=800000) on 8 TRN2 NeuronCores.

Strategy (edge parallelism per sharding hint):
- dst-shard nodes across 8 cores (6250 each); each core owns the edges into
  its shard, sorted by dst, padded to a uniform per-dst-tile chunk count so
  one SPMD program serves all cores.
- Phase A is *node-sharded*: core c computes the layer-1 table rows for its
  own 6250 nodes only ([SH, 272] = [h1 (256) | exp(a_src) (8) |
  exp(0.2 a_src) (8)]) using exp(leakyrelu(u+v)) = max(e^u e^v, e^.2u e^.2v),
  then an AllGather replicates the full [N, 272] table. The dst-side exp
  table adt1 [SH, 16] stays local (dst always lands in the owner's shard).
- L1 edge pass: per 128-edge chunk, indirect-DMA gather of h1x rows by src;
  attention weights via gathered exps x St-matmul-expanded dst exps; weighted
  scatter-add into per-dst-tile PSUM via one-hot S matmul (S built on device
  from compact dst-local bytes).
- AllGather of compact layer-2 table [6250,10] -> [50000,10]; L2 edge pass
  identical in structure; log_softmax epilogue.

I/O strategy (the axon tunnel is ~84 MB/s with ~10ms per shard transfer, so
bytes and array count dominate wall time): features are uploaded *sharded*
(2 MB/core instead of 16 MB/core replicated) as one bf16 array, and all
remaining per-core data (edge chunk tables, weights, biases, iota/identity
constants) is packed into ONE int32 blob per core, with bf16 sections read
on device via bitcast APs. Uploads are issued asynchronously so the feature
transfer overlaps the host-side edge bucketing.
"""
import numpy as np
import ml_dtypes

import jax
from jax.sharding import Mesh, NamedSharding, PartitionSpec
from jax.experimental.shard_map import shard_map

import concourse.bass as bass
import concourse.mybir as mybir
import concourse.tile as tile
from concourse import bacc

BF = ml_dtypes.bfloat16
P = 128
NCORES = 8
N = 50000
SH = N // NCORES          # 6250 nodes per core
NT = (SH + P - 1) // P    # 49 dst tiles per core
LAST_ROWS = SH - (NT - 1) * P  # 106
HIGH, LOW, EMB = 128, 32, 64
IN1 = HIGH + EMB
HID, HEADS, OUT = 32, 8, 8
IN2 = HID * HEADS
B = 16                    # chunks per super-chunk
NEG = 0.2
FROWS = HIGH + LOW + 1    # feature blob rows per core: highT | lowT | ones

AF = mybir.ActivationFunctionType
ALU = mybir.AluOpType


# ---------------------------------------------------------------- blob layout
def _blob_layout(NSUP):
    """int32 blob: [SRCG i32 | DSTT i32 | bf16 sections (bitcast)]."""
    S1 = NSUP * P * B
    S2 = NT * P
    secs = {}
    off = S1 + S2
    for name, shape in [
        ("DL", (NSUP, P, B)),
        ("W1t", (HIGH, 272)),
        ("W1b", (EMB, 272)),
        ("Wemb", (LOW + 1, EMB)),
        ("W2e", (P, 2, 10)),
        ("idn", (P, P)),
        ("iot", (P, P)),
        ("b1b", (P, IN2)),
        ("b2b", (P, OUT)),
    ]:
        n = int(np.prod(shape))
        assert n % 2 == 0
        secs[name] = (off, n, shape)
        off += n // 2
    return S1, S2, secs, off


# ---------------------------------------------------------------- host prep
_ONES_ROW = np.ones((NCORES, 1, SH), BF)


def _prep_feat(inputs):
    """[8*FROWS, SH] bf16: per core rows = [high^T (128) | low^T (32) | ones]."""
    high = np.asarray(inputs["high_dim_features"], np.float32)
    low = np.asarray(inputs["low_dim_features"], np.float32)
    hp = high.reshape(NCORES, SH, HIGH).transpose(0, 2, 1).astype(BF)
    lp = low.reshape(NCORES, SH, LOW).transpose(0, 2, 1).astype(BF)
    FG = np.concatenate([hp, lp, _ONES_ROW], axis=1)
    return FG.reshape(NCORES * FROWS, SH)


def _prep_edges(inputs):
    """Bucket edges by (dst core, dst tile) into 128-edge chunks; pack blob."""
    ei = np.asarray(inputs["edge_index"])
    loops = np.arange(N, dtype=np.int32)
    src = np.concatenate([ei[0].astype(np.int32), loops])
    dst = np.concatenate([ei[1].astype(np.int32), loops])
    # pack (tile id 9b | src 16b | dst%P 7b) into int32; one radix sort
    # replaces the stable argsort (in-bucket order is irrelevant)
    dlg = dst % SH
    tg0 = ((dst // SH) * NT + dlg // P).astype(np.uint32)
    key = np.sort((tg0 << np.uint32(23))
                  | (src.astype(np.uint32) << np.uint32(7))
                  | (dlg % P).astype(np.uint32))
    tg = (key >> np.uint32(23)).astype(np.int64)
    ss = ((key >> np.uint32(7)) & np.uint32(0xFFFF)).astype(np.int32)
    dlp = (key & np.uint32(0x7F)).astype(np.int32)
    starts = np.searchsorted(tg, np.arange(NCORES * NT + 1))
    cnt = np.diff(starts).reshape(NCORES, NT)
    C_t = np.maximum(1, np.ceil(cnt.max(axis=0) / P).astype(np.int64))
    TC = int(C_t.sum())
    NSUP = (TC + B - 1) // B
    # pad the last tile's chunk range to the full NSUP*B slot count so the
    # device loop is uniform (pad chunks have dl=-1 -> zero one-hot)
    C_t[NT - 1] += NSUP * B - TC
    TC = NSUP * B
    first_chunk = np.concatenate([[0], np.cumsum(C_t)])[:NT]
    tile_of_chunk = np.repeat(np.arange(NT), C_t)

    pos = np.arange(len(tg)) - starts[tg]
    core = tg // NT
    tloc = tg - core * NT
    flat = (core.astype(np.int64) * TC + first_chunk[tloc] + pos // P) * P \
        + pos % P
    srcg = np.zeros((NCORES, TC, P), np.int32)
    dlc = np.full((NCORES, TC, P), -1.0, BF)
    srcg.reshape(-1)[flat] = ss
    dlc.reshape(-1)[flat] = dlp.astype(np.float32)
    srcg_dev = np.ascontiguousarray(
        srcg.reshape(NCORES, NSUP, B, P).transpose(0, 1, 3, 2))  # [8,NSUP,P,B]
    dl_dev = np.ascontiguousarray(
        dlc.reshape(NCORES, NSUP, B, P).transpose(0, 1, 3, 2))

    S1, S2, secs, Lr = _blob_layout(NSUP)
    blob = np.empty((NCORES, Lr), np.int32)
    blob[:, :S1] = srcg_dev.reshape(NCORES, S1)
    dstt = np.minimum(np.arange(NT)[:, None] * P + np.arange(P)[None, :],
                      SH - 1).astype(np.int32)                   # local ids
    blob[:, S1:S1 + S2] = dstt.reshape(-1)[None, :]
    off, n, _ = secs["DL"]
    blob[:, off:off + n // 2] = dl_dev.reshape(NCORES, n).view(np.int32)

    # weight folding (replicated across cores)
    W1 = np.asarray(inputs["W1"], np.float32)          # [192, 256]
    as1 = np.asarray(inputs["att_src1"], np.float32)
    ad1 = np.asarray(inputs["att_dst1"], np.float32)
    W1As = np.einsum("khj,hj->kh", W1.reshape(IN1, HEADS, HID), as1)
    W1Ad = np.einsum("khj,hj->kh", W1.reshape(IN1, HEADS, HID), ad1)
    W1ex = np.concatenate([W1, W1As, W1Ad], axis=1).astype(BF)   # [192, 272]
    W2 = np.asarray(inputs["W2"], np.float32)          # [256, 8]
    W2As = W2 @ np.asarray(inputs["att_src2"], np.float32).reshape(OUT, 1)
    W2Ad = W2 @ np.asarray(inputs["att_dst2"], np.float32).reshape(OUT, 1)
    W2ex = np.concatenate([W2, W2As, W2Ad], axis=1).astype(BF)   # [256, 10]
    Wemb = np.asarray(inputs["W_emb"], np.float32)
    Wemb1 = np.concatenate(
        [Wemb, np.asarray(inputs["b_emb"], np.float32)[None, :]],
        axis=0).astype(BF)                             # [33, 64]
    wparts = {
        "W1t": np.ascontiguousarray(W1ex[:HIGH]),
        "W1b": np.ascontiguousarray(W1ex[HIGH:]),
        "Wemb": Wemb1,
        "W2e": np.ascontiguousarray(
            W2ex.reshape(2, P, 10).transpose(1, 0, 2)),          # [P, 2, 10]
        "idn": np.eye(P, dtype=np.float32).astype(BF),
        "iot": np.broadcast_to(np.arange(P, dtype=np.float32),
                               (P, P)).astype(BF).copy(),
        "b1b": np.broadcast_to(np.asarray(inputs["b1"], np.float32),
                               (P, IN2)).astype(BF).copy(),
        "b2b": np.broadcast_to(np.asarray(inputs["b2"], np.float32),
                               (P, OUT)).astype(BF).copy(),
    }
    for name, arr in wparts.items():
        off, n, shape = secs[name]
        assert arr.shape == shape, (name, arr.shape, shape)
        blob[:, off:off + n // 2] = arr.reshape(-1).view(np.int32)[None, :]

    return blob, C_t, TC, NSUP, tile_of_chunk, first_chunk, Lr


# ---------------------------------------------------------------- device build
def _build(C_t, TC, NSUP, tile_of_chunk, first_chunk, Lr):
    nc = bacc.Bacc("TRN2", target_bir_lowering=False, debug=False,
                   num_devices=NCORES)
    bf, f32, i32 = mybir.dt.bfloat16, mybir.dt.float32, mybir.dt.int32

    FEAT = nc.dram_tensor("FEAT", [FROWS, SH], bf, kind="ExternalInput")
    BLOB = nc.dram_tensor("BLOB", [Lr], i32, kind="ExternalInput")
    out_d = nc.dram_tensor("out", [SH, OUT], f32, kind="ExternalOutput")

    adt1 = nc.dram_tensor("adt1", [SH, 16], bf)
    adt2 = nc.dram_tensor("adt2", [SH, 2], bf)

    S1, S2, secs, Lr2 = _blob_layout(NSUP)
    assert Lr2 == Lr

    def sec_ap(name):
        off, n, shape = secs[name]
        ap = BLOB[off:off + n // 2].bitcast(bf)
        if len(shape) == 2:
            return ap.rearrange("(a b) -> a b", b=shape[1])
        return ap.rearrange("(a b c) -> a b c", b=shape[1], c=shape[2])

    def srcg_ap(s):
        return BLOB[s * P * B:(s + 1) * P * B].rearrange("(p b) -> p b", b=B)

    def dl_ap(s):
        off = secs["DL"][0]
        return BLOB[off + s * P * B // 2:off + (s + 1) * P * B // 2] \
            .bitcast(bf).rearrange("(p b) -> p b", b=B)

    def dstt_ap(t):
        return BLOB[S1 + t * P:S1 + (t + 1) * P].rearrange("(p a) -> p a", a=1)

    with tile.TileContext(nc) as tc:
        with tc.tile_pool(name="const", bufs=1) as cpool, \
             tc.tile_pool(name="sb", bufs=3) as sb, \
             tc.tile_pool(name="gat", bufs=3) as gat, \
             tc.tile_pool(name="psA", bufs=2, space="PSUM") as psA, \
             tc.tile_pool(name="psB", bufs=3, space="PSUM") as psB, \
             tc.tile_pool(name="dram", bufs=1, space="DRAM") as dram:

            h1l = dram.tile([SH, 272], bf)
            h1x = dram.tile([N, 272], bf)

            w1t = cpool.tile([HIGH, 272], bf)
            nc.sync.dma_start(out=w1t[:], in_=sec_ap("W1t"))
            w1b = cpool.tile([EMB, 272], bf)
            nc.sync.dma_start(out=w1b[:], in_=sec_ap("W1b"))
            wem = cpool.tile([LOW + 1, EMB], bf)
            nc.sync.dma_start(out=wem[:], in_=sec_ap("Wemb"))
            w2e = cpool.tile([P, 2, 10], bf)
            nc.sync.dma_start(out=w2e[:], in_=sec_ap("W2e"))
            b1s = cpool.tile([P, IN2], bf)
            nc.sync.dma_start(out=b1s[:], in_=sec_ap("b1b"))
            b2s = cpool.tile([P, OUT], bf)
            nc.sync.dma_start(out=b2s[:], in_=sec_ap("b2b"))
            ids = cpool.tile([P, P], bf)
            nc.sync.dma_start(out=ids[:], in_=sec_ap("idn"))
            iot = cpool.tile([P, P], bf)
            nc.sync.dma_start(out=iot[:], in_=sec_ap("iot"))

            # -------- Phase A: layer-1 tables for this core's SH nodes -------
            for ntile in range(NT):
                n0 = ntile * P
                w = min(P, SH - n0)
                ht = sb.tile([P, P], bf, tag="ht")
                nc.sync.dma_start(out=ht[:, :w], in_=FEAT[0:HIGH, n0:n0 + w])
                lt = sb.tile([LOW + 1, P], bf, tag="lt")
                nc.sync.dma_start(out=lt[:, :w], in_=FEAT[HIGH:FROWS, n0:n0 + w])
                embp = psB.tile([EMB, P], f32, tag="pB")
                nc.tensor.matmul(out=embp[:, :w], lhsT=wem[:], rhs=lt[:, :w],
                                 start=True, stop=True)
                # elu(v) = max(v,0)-1 + exp(-relu(-v))
                tm = sb.tile([EMB, P], f32, tag="tm")
                nc.scalar.activation(tm[:, :w], embp[:, :w], AF.Relu, scale=-1.0)
                te = sb.tile([EMB, P], f32, tag="te")
                nc.scalar.activation(te[:, :w], tm[:, :w], AF.Exp, scale=-1.0)
                tr = sb.tile([EMB, P], f32, tag="tr")
                nc.vector.tensor_scalar(tr[:, :w], embp[:, :w], 0.0, -1.0,
                                        ALU.max, ALU.add)
                embs = sb.tile([EMB, P], bf, tag="embs")
                nc.vector.tensor_tensor(embs[:, :w], tr[:, :w], te[:, :w],
                                        ALU.add)
                h1p = psA.tile([P, 512], f32, tag="pA")
                nc.tensor.matmul(out=h1p[:w, 0:272], lhsT=ht[:, :w], rhs=w1t[:],
                                 start=True, stop=False)
                nc.tensor.matmul(out=h1p[:w, 0:272], lhsT=embs[:, :w],
                                 rhs=w1b[:], start=False, stop=True)
                h1s = sb.tile([P, 272], bf, tag="h1s")
                nc.scalar.activation(h1s[:w, 0:256], h1p[:w, 0:256], AF.Copy)
                ads = sb.tile([P, 16], bf, tag="ads")
                nc.scalar.activation(h1s[:w, 256:264], h1p[:w, 256:264], AF.Exp)
                nc.scalar.activation(h1s[:w, 264:272], h1p[:w, 256:264], AF.Exp,
                                     scale=NEG)
                nc.scalar.activation(ads[:w, 0:8], h1p[:w, 264:272], AF.Exp)
                nc.scalar.activation(ads[:w, 8:16], h1p[:w, 264:272], AF.Exp,
                                     scale=NEG)
                nc.sync.dma_start(out=h1l[n0:n0 + w, :], in_=h1s[:w])
                nc.sync.dma_start(out=adt1[n0:n0 + w, :], in_=ads[:w])

            # -------- replicate the layer-1 table --------
            nc.gpsimd.collective_compute(
                "AllGather", ALU.bypass,
                replica_groups=[list(range(NCORES))],
                ins=[h1l.opt()], outs=[h1x.opt()])

            # ---------------- L1 edge pass ----------------
            h2xl = dram.tile([SH, 10], bf)
            h2xf = dram.tile([N, 10], bf)

            acc_of_tile = {}
            adt_of_tile = {}

            def l1_epilogue(t):
                rows = P if t < NT - 1 else LAST_ROWS
                acc = acc_of_tile.pop(t)
                rz = sb.tile([P, 8], f32, tag="rz")
                nc.vector.reciprocal(rz[:rows], acc[:rows, 256:264])
                xr = sb.tile([P, IN2], f32, tag="xr")
                nc.vector.tensor_tensor(
                    xr[:rows],
                    acc[:rows, 0:256].rearrange("p (h j) -> p h j", j=HID),
                    rz[:rows, :, None].to_broadcast([rows, 8, HID]), ALU.mult)
                nc.vector.tensor_tensor(xr[:rows], xr[:rows], b1s[:rows],
                                        ALU.add)
                tm = sb.tile([P, IN2], f32, tag="etm")
                nc.scalar.activation(tm[:rows], xr[:rows], AF.Relu, scale=-1.0)
                te = sb.tile([P, IN2], f32, tag="ete")
                nc.scalar.activation(te[:rows], tm[:rows], AF.Exp, scale=-1.0)
                tr = sb.tile([P, IN2], f32, tag="etr")
                nc.vector.tensor_scalar(tr[:rows], xr[:rows], 0.0, -1.0,
                                        ALU.max, ALU.add)
                x2 = sb.tile([P, IN2], bf, tag="x2")
                if rows < P:
                    nc.vector.memset(x2[:], 0.0)
                nc.vector.tensor_tensor(x2[:rows], tr[:rows], te[:rows],
                                        ALU.add)
                # x2T blocks + h2x row
                x2tb = sb.tile([P, 2, P], bf, tag="x2tb")
                for k in range(2):
                    tp = psB.tile([P, P], bf, tag="pB")
                    nc.tensor.transpose(out=tp[:], in_=x2[:, k * P:(k + 1) * P],
                                        identity=ids[:])
                    nc.vector.tensor_copy(x2tb[:, k, :], tp[:])
                h2p = psB.tile([P, 16], f32, tag="pB")
                for k in range(2):
                    nc.tensor.matmul(out=h2p[:, 0:10], lhsT=x2tb[:, k, :],
                                     rhs=w2e[:, k, :], start=(k == 0),
                                     stop=(k == 1))
                h2r = sb.tile([P, 10], bf, tag="h2r")
                nc.scalar.activation(h2r[:rows, 0:8], h2p[:rows, 0:8], AF.Copy)
                nc.scalar.activation(h2r[:rows, 8:9], h2p[:rows, 8:9], AF.Exp)
                nc.scalar.activation(h2r[:rows, 9:10], h2p[:rows, 8:9], AF.Exp,
                                     scale=NEG)
                a2r = sb.tile([P, 2], bf, tag="a2r")
                nc.scalar.activation(a2r[:rows, 0:1], h2p[:rows, 9:10], AF.Exp)
                nc.scalar.activation(a2r[:rows, 1:2], h2p[:rows, 9:10], AF.Exp,
                                     scale=NEG)
                nc.sync.dma_start(out=h2xl[t * P:t * P + rows, :], in_=h2r[:rows])
                nc.sync.dma_start(out=adt2[t * P:t * P + rows, :], in_=a2r[:rows])

            for s in range(NSUP):
                c0 = s * B
                it = gat.tile([P, B], i32, tag="it")
                nc.sync.dma_start(out=it[:], in_=srcg_ap(s))
                dlt = gat.tile([P, B], bf, tag="dlt")
                nc.sync.dma_start(out=dlt[:], in_=dl_ap(s))
                ssb = gat.tile([P, B * P], bf, tag="ssb")
                nc.vector.tensor_tensor(
                    ssb[:].rearrange("p (b q) -> p b q", q=P),
                    dlt[:, :, None].to_broadcast([P, B, P]),
                    iot[:, None, :].to_broadcast([P, B, P]), ALU.is_equal)
                sts = gat.tile([P, B * P], bf, tag="sts")
                for ci in range(B):
                    tpp = psB.tile([P, P], bf, tag="pB", name=f"stp{ci}")
                    nc.tensor.transpose(out=tpp[:],
                                        in_=ssb[:, ci * P:(ci + 1) * P],
                                        identity=ids[:])
                    nc.scalar.activation(sts[:, ci * P:(ci + 1) * P], tpp[:],
                                         AF.Copy)
                hg = gat.tile([P, B, 272], bf, tag="hg")
                adp = psB.tile([P, B * 16], f32, tag="pAD")
                for ci in range(B):
                    c = c0 + ci
                    t = int(tile_of_chunk[c])
                    if c == int(first_chunk[t]):
                        dtt = sb.tile([P, 1], i32, tag="dtt")
                        nc.sync.dma_start(out=dtt[:], in_=dstt_ap(t))
                        adtt = sb.tile([P, 16], bf, tag=f"adtt{t % 3}")
                        nc.gpsimd.indirect_dma_start(
                            out=adtt[:], out_offset=None, in_=adt1[:],
                            in_offset=bass.IndirectOffsetOnAxis(
                                ap=dtt[:, :1], axis=0))
                        adt_of_tile[t] = adtt
                        acc_of_tile[t] = psA.tile([P, 512], f32, tag="pA",
                                                  name=f"acc{t}")
                    nc.gpsimd.indirect_dma_start(
                        out=hg[:, ci, :], out_offset=None, in_=h1x[:],
                        in_offset=bass.IndirectOffsetOnAxis(
                            ap=it[:, ci:ci + 1], axis=0))
                    nc.tensor.matmul(out=adp[:, ci * 16:(ci + 1) * 16],
                                     lhsT=sts[:, ci * P:(ci + 1) * P],
                                     rhs=adt_of_tile[t][:], start=True,
                                     stop=True)
                # batched attention weights
                t1 = gat.tile([P, B * 8], f32, tag="t1")
                nc.vector.tensor_tensor(
                    t1[:].rearrange("p (b h) -> p b h", h=8),
                    hg[:, :, 256:264],
                    adp[:].rearrange("p (b h) -> p b h", h=16)[:, :, 0:8],
                    ALU.mult)
                t2 = gat.tile([P, B * 8], f32, tag="t2")
                nc.vector.tensor_tensor(
                    t2[:].rearrange("p (b h) -> p b h", h=8),
                    hg[:, :, 264:272],
                    adp[:].rearrange("p (b h) -> p b h", h=16)[:, :, 8:16],
                    ALU.mult)
                nc.vector.tensor_tensor(
                    hg[:, :, 256:264],
                    t1[:].rearrange("p (b h) -> p b h", h=8),
                    t2[:].rearrange("p (b h) -> p b h", h=8),
                    ALU.max)
                nc.vector.tensor_tensor(
                    hg[:, :, 0:256].rearrange("p b (h j) -> p b h j", j=HID),
                    hg[:, :, 0:256].rearrange("p b (h j) -> p b h j", j=HID),
                    hg[:, :, 256:264][:, :, :, None].to_broadcast(
                        [P, B, 8, HID]),
                    ALU.mult)
                for ci in range(B):
                    c = c0 + ci
                    t = int(tile_of_chunk[c])
                    last = (c == int(first_chunk[t]) + int(C_t[t]) - 1)
                    nc.tensor.matmul(out=acc_of_tile[t][:, 0:264],
                                     lhsT=ssb[:, ci * P:(ci + 1) * P],
                                     rhs=hg[:, ci, 0:264],
                                     start=(c == int(first_chunk[t])),
                                     stop=last)
                    if last:
                        l1_epilogue(t)

            # ---------------- AllGather layer-2 table ----------------
            nc.gpsimd.collective_compute(
                "AllGather", ALU.bypass,
                replica_groups=[list(range(NCORES))],
                ins=[h2xl.opt()], outs=[h2xf.opt()])

            # ---------------- L2 edge pass ----------------
            acc2_of_tile = {}
            adt2_of_tile = {}

            def l2_epilogue(t):
                rows = P if t < NT - 1 else LAST_ROWS
                acc = acc2_of_tile.pop(t)
                rz = sb.tile([P, 1], f32, tag="rz2")
                nc.vector.reciprocal(rz[:rows], acc[:rows, 8:9])
                o = sb.tile([P, OUT], f32, tag="o2")
                nc.vector.tensor_tensor(
                    o[:rows], acc[:rows, 0:8],
                    rz[:rows, :].to_broadcast([rows, OUT]), ALU.mult)
                nc.vector.tensor_tensor(o[:rows], o[:rows], b2s[:rows], ALU.add)
                ex = sb.tile([P, OUT], f32, tag="ex2")
                nc.scalar.activation(ex[:rows], o[:rows], AF.Exp)
                sm = sb.tile([P, 1], f32, tag="sm2")
                nc.vector.reduce_sum(sm[:rows], ex[:rows],
                                     axis=mybir.AxisListType.X)
                lg = sb.tile([P, 1], f32, tag="lg2")
                nc.scalar.activation(lg[:rows], sm[:rows], AF.Ln)
                fo = sb.tile([P, OUT], f32, tag="fo2")
                nc.vector.tensor_tensor(
                    fo[:rows], o[:rows],
                    lg[:rows, :].to_broadcast([rows, OUT]), ALU.subtract)
                nc.sync.dma_start(out=out_d[t * P:t * P + rows, :], in_=fo[:rows])

            for s in range(NSUP):
                c0 = s * B
                it = gat.tile([P, B], i32, tag="it")
                nc.sync.dma_start(out=it[:], in_=srcg_ap(s))
                dlt = gat.tile([P, B], bf, tag="dlt")
                nc.sync.dma_start(out=dlt[:], in_=dl_ap(s))
                ssb = gat.tile([P, B * P], bf, tag="ssb")
                nc.vector.tensor_tensor(
                    ssb[:].rearrange("p (b q) -> p b q", q=P),
                    dlt[:, :, None].to_broadcast([P, B, P]),
                    iot[:, None, :].to_broadcast([P, B, P]), ALU.is_equal)
                sts = gat.tile([P, B * P], bf, tag="sts")
                for ci in range(B):
                    tpp = psB.tile([P, P], bf, tag="pB", name=f"stp{ci}")
                    nc.tensor.transpose(out=tpp[:],
                                        in_=ssb[:, ci * P:(ci + 1) * P],
                                        identity=ids[:])
                    nc.scalar.activation(sts[:, ci * P:(ci + 1) * P], tpp[:],
                                         AF.Copy)
                hg2 = gat.tile([P, B, 10], bf, tag="hg2")
                adp2 = psB.tile([P, B * 2], f32, tag="pAD")
                for ci in range(B):
                    c = c0 + ci
                    t = int(tile_of_chunk[c])
                    if c == int(first_chunk[t]):
                        a2t = sb.tile([P, 2], bf, tag=f"a2t{t % 3}")
                        rows = P if t < NT - 1 else LAST_ROWS
                        if rows < P:
                            nc.vector.memset(a2t[:], 0.0)
                        nc.sync.dma_start(out=a2t[:rows],
                                          in_=adt2[t * P:t * P + rows, :])
                        adt2_of_tile[t] = a2t
                        acc2_of_tile[t] = psA.tile([P, 512], f32, tag="pA",
                                                   name=f"acc2_{t}")
                    nc.gpsimd.indirect_dma_start(
                        out=hg2[:, ci, :], out_offset=None, in_=h2xf[:],
                        in_offset=bass.IndirectOffsetOnAxis(
                            ap=it[:, ci:ci + 1], axis=0))
                    nc.tensor.matmul(out=adp2[:, ci * 2:(ci + 1) * 2],
                                     lhsT=sts[:, ci * P:(ci + 1) * P],
                                     rhs=adt2_of_tile[t][:], start=True,
                                     stop=True)
                t1 = gat.tile([P, B], f32, tag="t1b")
                nc.vector.tensor_tensor(
                    t1[:, :, None], hg2[:, :, 8:9],
                    adp2[:].rearrange("p (b k) -> p b k", k=2)[:, :, 0:1],
                    ALU.mult)
                t2 = gat.tile([P, B], f32, tag="t2b")
                nc.vector.tensor_tensor(
                    t2[:, :, None], hg2[:, :, 9:10],
                    adp2[:].rearrange("p (b k) -> p b k", k=2)[:, :, 1:2],
                    ALU.mult)
                nc.vector.tensor_tensor(
                    hg2[:, :, 8:9], t1[:, :, None], t2[:, :, None], ALU.max)
                nc.vector.tensor_tensor(
                    hg2[:, :, 0:8], hg2[:, :, 0:8],
                    hg2[:, :, 8:9].to_broadcast([P, B, OUT]), ALU.mult)
                for ci in range(B):
                    c = c0 + ci
                    t = int(tile_of_chunk[c])
                    last = (c == int(first_chunk[t]) + int(C_t[t]) - 1)
                    nc.tensor.matmul(out=acc2_of_tile[t][:, 0:9],
                                     lhsT=ssb[:, ci * P:(ci + 1) * P],
                                     rhs=hg2[:, ci, 0:9],
                                     start=(c == int(first_chunk[t])),
                                     stop=last)
                    if last:
                        l2_epilogue(t)

    if not nc.is_finalized():
        nc.finalize()
    return nc


# ---------------------------------------------------------------- runner
_CACHE = {}   # structure key -> (nc, runner)


def _make_runner(nc):
    """Cached-jit replica of bass2jax.run_bass_via_pjrt (axon path)."""
    from concourse.bass2jax import (install_neuronx_cc_hook,
                                    partition_id_tensor, _bass_exec_p)
    install_neuronx_cc_hook()
    partition_name = (nc.partition_id_tensor.name
                      if nc.partition_id_tensor else None)
    in_names, out_names, out_avals = [], [], []
    for alloc in nc.m.functions[0].allocations:
        if not isinstance(alloc, mybir.MemoryLocationSet):
            continue
        name = alloc.memorylocations[0].name
        if alloc.kind == "ExternalInput":
            if name != partition_name:
                in_names.append(name)
        elif alloc.kind == "ExternalOutput":
            out_names.append(name)
            out_avals.append(jax.core.ShapedArray(
                tuple(alloc.tensor_shape), mybir.dt.np(alloc.dtype)))
    n_params = len(in_names)
    n_outs = len(out_avals)
    in_names_all = in_names + out_names + (
        [partition_name] if partition_name else [])
    donate = tuple(range(n_params, n_params + n_outs))

    def _body(*args):
        operands = list(args)
        if partition_name is not None:
            operands.append(partition_id_tensor())
        outs = _bass_exec_p.bind(
            *operands, out_avals=tuple(out_avals),
            in_names=tuple(in_names_all), out_names=tuple(out_names),
            lowering_input_output_aliases=(), sim_require_finite=True,
            sim_require_nnan=True, nc=nc)
        return tuple(outs)

    devices = jax.devices()[:NCORES]
    mesh = Mesh(np.asarray(devices), ("core",))
    sharding = NamedSharding(mesh, PartitionSpec("core"))
    in_specs = (PartitionSpec("core"),) * (n_params + n_outs)
    out_specs = (PartitionSpec("core"),) * len(out_names)
    fn = jax.jit(
        shard_map(_body, mesh=mesh, in_specs=in_specs, out_specs=out_specs,
                  check_rep=False),
        donate_argnums=donate, keep_unused=True)
    return fn, in_names, out_names, out_avals, sharding


def kernel(**inputs):
    FG = _prep_feat(inputs)
    # fire the (dominant) feature upload before doing edge bucketing so the
    # tunnel transfer overlaps host prep
    dfeat = dzeros = None
    try:
        devices = jax.devices()[:NCORES]
        mesh = Mesh(np.asarray(devices), ("core",))
        sharding = NamedSharding(mesh, PartitionSpec("core"))
        dfeat = jax.device_put(FG, sharding)
        dzeros = jax.device_put(
            np.zeros((NCORES * SH, OUT), np.float32), sharding)
    except Exception:
        dfeat = dzeros = None

    blob, C_t, TC, NSUP, toc, fc, Lr = _prep_edges(inputs)
    key = (TC, NSUP, tuple(int(x) for x in C_t))
    if key not in _CACHE:
        nc = _build(C_t, TC, NSUP, toc, fc, Lr)
        _CACHE[key] = (nc, _make_runner(nc))
    nc, (fn, in_names, out_names, out_avals, sharding) = _CACHE[key]

    host_in = {"FEAT": FG, "BLOB": blob.reshape(-1)}
    dev_in = []
    for name in in_names:
        if name == "FEAT" and dfeat is not None:
            dev_in.append(dfeat)
        else:
            dev_in.append(jax.device_put(host_in[name], sharding))
    if dzeros is None:
        dzeros = jax.device_put(
            np.zeros((NCORES * SH, OUT), np.float32), sharding)
    out_arrs = fn(*dev_in, dzeros)
    out = np.asarray(out_arrs[out_names.index("out")])
    return out.reshape(N, OUT)


# revision 12
# speedup vs baseline: 12.3916x; 1.0136x over previous
"""CombinedGAT (2-layer GAT, N=50000, E=800000) on 8 TRN2 NeuronCores.

Strategy (edge parallelism per sharding hint):
- dst-shard nodes across 8 cores (6250 each); each core owns the edges into
  its shard, sorted by dst, padded to a uniform per-dst-tile chunk count so
  one SPMD program serves all cores.
- Phase A is *node-sharded*: core c computes the layer-1 table rows for its
  own 6250 nodes only ([SH, 272] = [h1 (256) | exp(a_src) (8) |
  exp(0.2 a_src) (8)]) using exp(leakyrelu(u+v)) = max(e^u e^v, e^.2u e^.2v),
  then an AllGather replicates the full [N, 272] table. The dst-side exp
  table adt1 [SH, 16] stays local (dst always lands in the owner's shard).
- L1 edge pass: per 128-edge chunk, indirect-DMA gather of h1x rows by src;
  attention weights via gathered exps x St-matmul-expanded dst exps; weighted
  scatter-add into per-dst-tile PSUM via one-hot S matmul (S built on device
  from compact dst-local bytes).
- AllGather of compact layer-2 table [6250,10] -> [50000,10]; L2 edge pass
  identical in structure; log_softmax epilogue.

I/O strategy (the axon tunnel is ~84 MB/s with ~10ms per shard transfer, so
bytes and array count dominate wall time): features are uploaded *sharded*
(2 MB/core instead of 16 MB/core replicated) as one bf16 array, and all
remaining per-core data (edge chunk tables, weights, biases, iota/identity
constants) is packed into ONE int32 blob per core, with bf16 sections read
on device via bitcast APs. Uploads are issued asynchronously so the feature
transfer overlaps the host-side edge bucketing.
"""
import numpy as np
import ml_dtypes

import jax
from jax.sharding import Mesh, NamedSharding, PartitionSpec
from jax.experimental.shard_map import shard_map

import concourse.bass as bass
import concourse.mybir as mybir
import concourse.tile as tile
from concourse import bacc

BF = ml_dtypes.bfloat16
F8 = ml_dtypes.float8_e4m3
P = 128
NCORES = 8
N = 50000
SH = N // NCORES          # 6250 nodes per core
NT = (SH + P - 1) // P    # 49 dst tiles per core
LAST_ROWS = SH - (NT - 1) * P  # 106
HIGH, LOW, EMB = 128, 32, 64
IN1 = HIGH + EMB
HID, HEADS, OUT = 32, 8, 8
IN2 = HID * HEADS
B = 16                    # chunks per super-chunk
NEG = 0.2
FROWS = HIGH + LOW + 1    # feature blob rows per core: highT | lowT | ones

AF = mybir.ActivationFunctionType
ALU = mybir.AluOpType


# ---------------------------------------------------------------- blob layout
def _blob_layout(NSUP):
    """int32 blob: [SRCG i32 | DSTT i32 | bf16 sections (bitcast)]."""
    S1 = NSUP * P * B
    S2 = NT * P
    secs = {}
    off = S1 + S2
    for name, shape in [
        ("DL", (NSUP, P, B)),
        ("W1t", (HIGH, 272)),
        ("W1b", (EMB, 272)),
        ("Wemb", (LOW + 1, EMB)),
        ("W2e", (P, 2, 10)),
        ("idn", (P, P)),
        ("iot", (P, P)),
        ("b1b", (P, IN2)),
        ("b2b", (P, OUT)),
    ]:
        n = int(np.prod(shape))
        assert n % 2 == 0
        secs[name] = (off, n, shape)
        off += n // 2
    return S1, S2, secs, off


# ---------------------------------------------------------------- host prep
_ONES_ROW = np.ones((NCORES, 1, SH), F8)


def _prep_feat(inputs):
    """[8*FROWS, SH] fp8: per core rows = [high^T (128) | low^T (32) | ones].

    fp8-e4m3 features add ~2e-3 rel-fro error on top of the kernel's ~4.5e-3
    (gate 2e-2) and halve the dominant tunnel upload."""
    high = np.asarray(inputs["high_dim_features"], np.float32)
    low = np.asarray(inputs["low_dim_features"], np.float32)
    hp = high.reshape(NCORES, SH, HIGH).transpose(0, 2, 1).astype(F8)
    lp = low.reshape(NCORES, SH, LOW).transpose(0, 2, 1).astype(F8)
    FG = np.concatenate([hp, lp, _ONES_ROW], axis=1)
    return FG.reshape(NCORES * FROWS, SH)


def _prep_edges(inputs):
    """Bucket edges by (dst core, dst tile) into 128-edge chunks; pack blob."""
    ei = np.asarray(inputs["edge_index"])
    loops = np.arange(N, dtype=np.int32)
    src = np.concatenate([ei[0].astype(np.int32), loops])
    dst = np.concatenate([ei[1].astype(np.int32), loops])
    # pack (tile id 9b | src 16b | dst%P 7b) into int32; one radix sort
    # replaces the stable argsort (in-bucket order is irrelevant)
    dlg = dst % SH
    tg0 = ((dst // SH) * NT + dlg // P).astype(np.uint32)
    key = np.sort((tg0 << np.uint32(23))
                  | (src.astype(np.uint32) << np.uint32(7))
                  | (dlg % P).astype(np.uint32))
    tg = (key >> np.uint32(23)).astype(np.int64)
    ss = ((key >> np.uint32(7)) & np.uint32(0xFFFF)).astype(np.int32)
    dlp = (key & np.uint32(0x7F)).astype(np.int32)
    starts = np.searchsorted(tg, np.arange(NCORES * NT + 1))
    cnt = np.diff(starts).reshape(NCORES, NT)
    C_t = np.maximum(1, np.ceil(cnt.max(axis=0) / P).astype(np.int64))
    TC = int(C_t.sum())
    NSUP = (TC + B - 1) // B
    # pad the last tile's chunk range to the full NSUP*B slot count so the
    # device loop is uniform (pad chunks have dl=-1 -> zero one-hot)
    C_t[NT - 1] += NSUP * B - TC
    TC = NSUP * B
    first_chunk = np.concatenate([[0], np.cumsum(C_t)])[:NT]
    tile_of_chunk = np.repeat(np.arange(NT), C_t)

    pos = np.arange(len(tg)) - starts[tg]
    core = tg // NT
    tloc = tg - core * NT
    flat = (core.astype(np.int64) * TC + first_chunk[tloc] + pos // P) * P \
        + pos % P
    srcg = np.zeros((NCORES, TC, P), np.int32)
    dlc = np.full((NCORES, TC, P), -1.0, BF)
    srcg.reshape(-1)[flat] = ss
    dlc.reshape(-1)[flat] = dlp.astype(np.float32)
    srcg_dev = np.ascontiguousarray(
        srcg.reshape(NCORES, NSUP, B, P).transpose(0, 1, 3, 2))  # [8,NSUP,P,B]
    dl_dev = np.ascontiguousarray(
        dlc.reshape(NCORES, NSUP, B, P).transpose(0, 1, 3, 2))

    S1, S2, secs, Lr = _blob_layout(NSUP)
    blob = np.empty((NCORES, Lr), np.int32)
    blob[:, :S1] = srcg_dev.reshape(NCORES, S1)
    dstt = np.minimum(np.arange(NT)[:, None] * P + np.arange(P)[None, :],
                      SH - 1).astype(np.int32)                   # local ids
    blob[:, S1:S1 + S2] = dstt.reshape(-1)[None, :]
    off, n, _ = secs["DL"]
    blob[:, off:off + n // 2] = dl_dev.reshape(NCORES, n).view(np.int32)

    # weight folding (replicated across cores)
    W1 = np.asarray(inputs["W1"], np.float32)          # [192, 256]
    as1 = np.asarray(inputs["att_src1"], np.float32)
    ad1 = np.asarray(inputs["att_dst1"], np.float32)
    W1As = np.einsum("khj,hj->kh", W1.reshape(IN1, HEADS, HID), as1)
    W1Ad = np.einsum("khj,hj->kh", W1.reshape(IN1, HEADS, HID), ad1)
    W1ex = np.concatenate([W1, W1As, W1Ad], axis=1).astype(BF)   # [192, 272]
    W2 = np.asarray(inputs["W2"], np.float32)          # [256, 8]
    W2As = W2 @ np.asarray(inputs["att_src2"], np.float32).reshape(OUT, 1)
    W2Ad = W2 @ np.asarray(inputs["att_dst2"], np.float32).reshape(OUT, 1)
    W2ex = np.concatenate([W2, W2As, W2Ad], axis=1).astype(BF)   # [256, 10]
    Wemb = np.asarray(inputs["W_emb"], np.float32)
    Wemb1 = np.concatenate(
        [Wemb, np.asarray(inputs["b_emb"], np.float32)[None, :]],
        axis=0).astype(BF)                             # [33, 64]
    wparts = {
        "W1t": np.ascontiguousarray(W1ex[:HIGH]),
        "W1b": np.ascontiguousarray(W1ex[HIGH:]),
        "Wemb": Wemb1,
        "W2e": np.ascontiguousarray(
            W2ex.reshape(2, P, 10).transpose(1, 0, 2)),          # [P, 2, 10]
        "idn": np.eye(P, dtype=np.float32).astype(BF),
        "iot": np.broadcast_to(np.arange(P, dtype=np.float32),
                               (P, P)).astype(BF).copy(),
        "b1b": np.broadcast_to(np.asarray(inputs["b1"], np.float32),
                               (P, IN2)).astype(BF).copy(),
        "b2b": np.broadcast_to(np.asarray(inputs["b2"], np.float32),
                               (P, OUT)).astype(BF).copy(),
    }
    for name, arr in wparts.items():
        off, n, shape = secs[name]
        assert arr.shape == shape, (name, arr.shape, shape)
        blob[:, off:off + n // 2] = arr.reshape(-1).view(np.int32)[None, :]

    return blob, C_t, TC, NSUP, tile_of_chunk, first_chunk, Lr


# ---------------------------------------------------------------- device build
def _build(C_t, TC, NSUP, tile_of_chunk, first_chunk, Lr):
    nc = bacc.Bacc("TRN2", target_bir_lowering=False, debug=False,
                   num_devices=NCORES)
    bf, f32, i32 = mybir.dt.bfloat16, mybir.dt.float32, mybir.dt.int32

    f8 = mybir.dt.float8e4
    FEAT = nc.dram_tensor("FEAT", [FROWS, SH], f8, kind="ExternalInput")
    BLOB = nc.dram_tensor("BLOB", [Lr], i32, kind="ExternalInput")
    out_d = nc.dram_tensor("out", [SH, OUT], f32, kind="ExternalOutput")

    adt1 = nc.dram_tensor("adt1", [SH, 16], bf)
    adt2 = nc.dram_tensor("adt2", [SH, 2], bf)

    S1, S2, secs, Lr2 = _blob_layout(NSUP)
    assert Lr2 == Lr

    def sec_ap(name):
        off, n, shape = secs[name]
        ap = BLOB[off:off + n // 2].bitcast(bf)
        if len(shape) == 2:
            return ap.rearrange("(a b) -> a b", b=shape[1])
        return ap.rearrange("(a b c) -> a b c", b=shape[1], c=shape[2])

    def srcg_ap(s):
        return BLOB[s * P * B:(s + 1) * P * B].rearrange("(p b) -> p b", b=B)

    def dl_ap(s):
        off = secs["DL"][0]
        return BLOB[off + s * P * B // 2:off + (s + 1) * P * B // 2] \
            .bitcast(bf).rearrange("(p b) -> p b", b=B)

    def dstt_ap(t):
        return BLOB[S1 + t * P:S1 + (t + 1) * P].rearrange("(p a) -> p a", a=1)

    with tile.TileContext(nc) as tc:
        with tc.tile_pool(name="const", bufs=1) as cpool, \
             tc.tile_pool(name="sb", bufs=3) as sb, \
             tc.tile_pool(name="gat", bufs=3) as gat, \
             tc.tile_pool(name="psA", bufs=2, space="PSUM") as psA, \
             tc.tile_pool(name="psB", bufs=3, space="PSUM") as psB, \
             tc.tile_pool(name="dram", bufs=1, space="DRAM") as dram:

            h1l = dram.tile([SH, 272], bf)
            h1x = dram.tile([N, 272], bf)

            w1t = cpool.tile([HIGH, 272], bf)
            nc.sync.dma_start(out=w1t[:], in_=sec_ap("W1t"))
            w1b = cpool.tile([EMB, 272], bf)
            nc.sync.dma_start(out=w1b[:], in_=sec_ap("W1b"))
            wem = cpool.tile([LOW + 1, EMB], bf)
            nc.sync.dma_start(out=wem[:], in_=sec_ap("Wemb"))
            w2e = cpool.tile([P, 2, 10], bf)
            nc.sync.dma_start(out=w2e[:], in_=sec_ap("W2e"))
            b1s = cpool.tile([P, IN2], bf)
            nc.sync.dma_start(out=b1s[:], in_=sec_ap("b1b"))
            b2s = cpool.tile([P, OUT], bf)
            nc.sync.dma_start(out=b2s[:], in_=sec_ap("b2b"))
            ids = cpool.tile([P, P], bf)
            nc.sync.dma_start(out=ids[:], in_=sec_ap("idn"))
            iot = cpool.tile([P, P], bf)
            nc.sync.dma_start(out=iot[:], in_=sec_ap("iot"))

            # -------- Phase A: layer-1 tables for this core's SH nodes -------
            for ntile in range(NT):
                n0 = ntile * P
                w = min(P, SH - n0)
                ht8 = sb.tile([P, P], f8, tag="ht8")
                nc.sync.dma_start(out=ht8[:, :w], in_=FEAT[0:HIGH, n0:n0 + w])
                lt8 = sb.tile([LOW + 1, P], f8, tag="lt8")
                nc.sync.dma_start(out=lt8[:, :w], in_=FEAT[HIGH:FROWS, n0:n0 + w])
                ht = sb.tile([P, P], bf, tag="ht")
                nc.scalar.activation(ht[:, :w], ht8[:, :w], AF.Copy)
                lt = sb.tile([LOW + 1, P], bf, tag="lt")
                nc.scalar.activation(lt[:, :w], lt8[:, :w], AF.Copy)
                embp = psB.tile([EMB, P], f32, tag="pB")
                nc.tensor.matmul(out=embp[:, :w], lhsT=wem[:], rhs=lt[:, :w],
                                 start=True, stop=True)
                # elu(v) = max(v,0)-1 + exp(-relu(-v))
                tm = sb.tile([EMB, P], f32, tag="tm")
                nc.scalar.activation(tm[:, :w], embp[:, :w], AF.Relu, scale=-1.0)
                te = sb.tile([EMB, P], f32, tag="te")
                nc.scalar.activation(te[:, :w], tm[:, :w], AF.Exp, scale=-1.0)
                tr = sb.tile([EMB, P], f32, tag="tr")
                nc.vector.tensor_scalar(tr[:, :w], embp[:, :w], 0.0, -1.0,
                                        ALU.max, ALU.add)
                embs = sb.tile([EMB, P], bf, tag="embs")
                nc.vector.tensor_tensor(embs[:, :w], tr[:, :w], te[:, :w],
                                        ALU.add)
                h1p = psA.tile([P, 512], f32, tag="pA")
                nc.tensor.matmul(out=h1p[:w, 0:272], lhsT=ht[:, :w], rhs=w1t[:],
                                 start=True, stop=False)
                nc.tensor.matmul(out=h1p[:w, 0:272], lhsT=embs[:, :w],
                                 rhs=w1b[:], start=False, stop=True)
                h1s = sb.tile([P, 272], bf, tag="h1s")
                nc.scalar.activation(h1s[:w, 0:256], h1p[:w, 0:256], AF.Copy)
                ads = sb.tile([P, 16], bf, tag="ads")
                nc.scalar.activation(h1s[:w, 256:264], h1p[:w, 256:264], AF.Exp)
                nc.scalar.activation(h1s[:w, 264:272], h1p[:w, 256:264], AF.Exp,
                                     scale=NEG)
                nc.scalar.activation(ads[:w, 0:8], h1p[:w, 264:272], AF.Exp)
                nc.scalar.activation(ads[:w, 8:16], h1p[:w, 264:272], AF.Exp,
                                     scale=NEG)
                nc.sync.dma_start(out=h1l[n0:n0 + w, :], in_=h1s[:w])
                nc.sync.dma_start(out=adt1[n0:n0 + w, :], in_=ads[:w])

            # -------- replicate the layer-1 table --------
            nc.gpsimd.collective_compute(
                "AllGather", ALU.bypass,
                replica_groups=[list(range(NCORES))],
                ins=[h1l.opt()], outs=[h1x.opt()])

            # ---------------- L1 edge pass ----------------
            h2xl = dram.tile([SH, 10], bf)
            h2xf = dram.tile([N, 10], bf)

            acc_of_tile = {}
            adt_of_tile = {}

            def l1_epilogue(t):
                rows = P if t < NT - 1 else LAST_ROWS
                acc = acc_of_tile.pop(t)
                rz = sb.tile([P, 8], f32, tag="rz")
                nc.vector.reciprocal(rz[:rows], acc[:rows, 256:264])
                xr = sb.tile([P, IN2], f32, tag="xr")
                nc.vector.tensor_tensor(
                    xr[:rows],
                    acc[:rows, 0:256].rearrange("p (h j) -> p h j", j=HID),
                    rz[:rows, :, None].to_broadcast([rows, 8, HID]), ALU.mult)
                nc.vector.tensor_tensor(xr[:rows], xr[:rows], b1s[:rows],
                                        ALU.add)
                tm = sb.tile([P, IN2], f32, tag="etm")
                nc.scalar.activation(tm[:rows], xr[:rows], AF.Relu, scale=-1.0)
                te = sb.tile([P, IN2], f32, tag="ete")
                nc.scalar.activation(te[:rows], tm[:rows], AF.Exp, scale=-1.0)
                tr = sb.tile([P, IN2], f32, tag="etr")
                nc.vector.tensor_scalar(tr[:rows], xr[:rows], 0.0, -1.0,
                                        ALU.max, ALU.add)
                x2 = sb.tile([P, IN2], bf, tag="x2")
                if rows < P:
                    nc.vector.memset(x2[:], 0.0)
                nc.vector.tensor_tensor(x2[:rows], tr[:rows], te[:rows],
                                        ALU.add)
                # x2T blocks + h2x row
                x2tb = sb.tile([P, 2, P], bf, tag="x2tb")
                for k in range(2):
                    tp = psB.tile([P, P], bf, tag="pB")
                    nc.tensor.transpose(out=tp[:], in_=x2[:, k * P:(k + 1) * P],
                                        identity=ids[:])
                    nc.vector.tensor_copy(x2tb[:, k, :], tp[:])
                h2p = psB.tile([P, 16], f32, tag="pB")
                for k in range(2):
                    nc.tensor.matmul(out=h2p[:, 0:10], lhsT=x2tb[:, k, :],
                                     rhs=w2e[:, k, :], start=(k == 0),
                                     stop=(k == 1))
                h2r = sb.tile([P, 10], bf, tag="h2r")
                nc.scalar.activation(h2r[:rows, 0:8], h2p[:rows, 0:8], AF.Copy)
                nc.scalar.activation(h2r[:rows, 8:9], h2p[:rows, 8:9], AF.Exp)
                nc.scalar.activation(h2r[:rows, 9:10], h2p[:rows, 8:9], AF.Exp,
                                     scale=NEG)
                a2r = sb.tile([P, 2], bf, tag="a2r")
                nc.scalar.activation(a2r[:rows, 0:1], h2p[:rows, 9:10], AF.Exp)
                nc.scalar.activation(a2r[:rows, 1:2], h2p[:rows, 9:10], AF.Exp,
                                     scale=NEG)
                nc.sync.dma_start(out=h2xl[t * P:t * P + rows, :], in_=h2r[:rows])
                nc.sync.dma_start(out=adt2[t * P:t * P + rows, :], in_=a2r[:rows])

            for s in range(NSUP):
                c0 = s * B
                it = gat.tile([P, B], i32, tag="it")
                nc.sync.dma_start(out=it[:], in_=srcg_ap(s))
                dlt = gat.tile([P, B], bf, tag="dlt")
                nc.sync.dma_start(out=dlt[:], in_=dl_ap(s))
                ssb = gat.tile([P, B * P], bf, tag="ssb")
                nc.vector.tensor_tensor(
                    ssb[:].rearrange("p (b q) -> p b q", q=P),
                    dlt[:, :, None].to_broadcast([P, B, P]),
                    iot[:, None, :].to_broadcast([P, B, P]), ALU.is_equal)
                sts = gat.tile([P, B * P], bf, tag="sts")
                for ci in range(B):
                    tpp = psB.tile([P, P], bf, tag="pB", name=f"stp{ci}")
                    nc.tensor.transpose(out=tpp[:],
                                        in_=ssb[:, ci * P:(ci + 1) * P],
                                        identity=ids[:])
                    nc.scalar.activation(sts[:, ci * P:(ci + 1) * P], tpp[:],
                                         AF.Copy)
                hg = gat.tile([P, B, 272], bf, tag="hg")
                adp = psB.tile([P, B * 16], f32, tag="pAD")
                for ci in range(B):
                    c = c0 + ci
                    t = int(tile_of_chunk[c])
                    if c == int(first_chunk[t]):
                        dtt = sb.tile([P, 1], i32, tag="dtt")
                        nc.sync.dma_start(out=dtt[:], in_=dstt_ap(t))
                        adtt = sb.tile([P, 16], bf, tag=f"adtt{t % 3}")
                        nc.gpsimd.indirect_dma_start(
                            out=adtt[:], out_offset=None, in_=adt1[:],
                            in_offset=bass.IndirectOffsetOnAxis(
                                ap=dtt[:, :1], axis=0))
                        adt_of_tile[t] = adtt
                        acc_of_tile[t] = psA.tile([P, 512], f32, tag="pA",
                                                  name=f"acc{t}")
                    nc.gpsimd.indirect_dma_start(
                        out=hg[:, ci, :], out_offset=None, in_=h1x[:],
                        in_offset=bass.IndirectOffsetOnAxis(
                            ap=it[:, ci:ci + 1], axis=0))
                    nc.tensor.matmul(out=adp[:, ci * 16:(ci + 1) * 16],
                                     lhsT=sts[:, ci * P:(ci + 1) * P],
                                     rhs=adt_of_tile[t][:], start=True,
                                     stop=True)
                # batched attention weights
                t1 = gat.tile([P, B * 8], f32, tag="t1")
                nc.vector.tensor_tensor(
                    t1[:].rearrange("p (b h) -> p b h", h=8),
                    hg[:, :, 256:264],
                    adp[:].rearrange("p (b h) -> p b h", h=16)[:, :, 0:8],
                    ALU.mult)
                t2 = gat.tile([P, B * 8], f32, tag="t2")
                nc.vector.tensor_tensor(
                    t2[:].rearrange("p (b h) -> p b h", h=8),
                    hg[:, :, 264:272],
                    adp[:].rearrange("p (b h) -> p b h", h=16)[:, :, 8:16],
                    ALU.mult)
                nc.vector.tensor_tensor(
                    hg[:, :, 256:264],
                    t1[:].rearrange("p (b h) -> p b h", h=8),
                    t2[:].rearrange("p (b h) -> p b h", h=8),
                    ALU.max)
                nc.vector.tensor_tensor(
                    hg[:, :, 0:256].rearrange("p b (h j) -> p b h j", j=HID),
                    hg[:, :, 0:256].rearrange("p b (h j) -> p b h j", j=HID),
                    hg[:, :, 256:264][:, :, :, None].to_broadcast(
                        [P, B, 8, HID]),
                    ALU.mult)
                for ci in range(B):
                    c = c0 + ci
                    t = int(tile_of_chunk[c])
                    last = (c == int(first_chunk[t]) + int(C_t[t]) - 1)
                    nc.tensor.matmul(out=acc_of_tile[t][:, 0:264],
                                     lhsT=ssb[:, ci * P:(ci + 1) * P],
                                     rhs=hg[:, ci, 0:264],
                                     start=(c == int(first_chunk[t])),
                                     stop=last)
                    if last:
                        l1_epilogue(t)

            # ---------------- AllGather layer-2 table ----------------
            nc.gpsimd.collective_compute(
                "AllGather", ALU.bypass,
                replica_groups=[list(range(NCORES))],
                ins=[h2xl.opt()], outs=[h2xf.opt()])

            # ---------------- L2 edge pass ----------------
            acc2_of_tile = {}
            adt2_of_tile = {}

            def l2_epilogue(t):
                rows = P if t < NT - 1 else LAST_ROWS
                acc = acc2_of_tile.pop(t)
                rz = sb.tile([P, 1], f32, tag="rz2")
                nc.vector.reciprocal(rz[:rows], acc[:rows, 8:9])
                o = sb.tile([P, OUT], f32, tag="o2")
                nc.vector.tensor_tensor(
                    o[:rows], acc[:rows, 0:8],
                    rz[:rows, :].to_broadcast([rows, OUT]), ALU.mult)
                nc.vector.tensor_tensor(o[:rows], o[:rows], b2s[:rows], ALU.add)
                ex = sb.tile([P, OUT], f32, tag="ex2")
                nc.scalar.activation(ex[:rows], o[:rows], AF.Exp)
                sm = sb.tile([P, 1], f32, tag="sm2")
                nc.vector.reduce_sum(sm[:rows], ex[:rows],
                                     axis=mybir.AxisListType.X)
                lg = sb.tile([P, 1], f32, tag="lg2")
                nc.scalar.activation(lg[:rows], sm[:rows], AF.Ln)
                fo = sb.tile([P, OUT], f32, tag="fo2")
                nc.vector.tensor_tensor(
                    fo[:rows], o[:rows],
                    lg[:rows, :].to_broadcast([rows, OUT]), ALU.subtract)
                nc.sync.dma_start(out=out_d[t * P:t * P + rows, :], in_=fo[:rows])

            for s in range(NSUP):
                c0 = s * B
                it = gat.tile([P, B], i32, tag="it")
                nc.sync.dma_start(out=it[:], in_=srcg_ap(s))
                dlt = gat.tile([P, B], bf, tag="dlt")
                nc.sync.dma_start(out=dlt[:], in_=dl_ap(s))
                ssb = gat.tile([P, B * P], bf, tag="ssb")
                nc.vector.tensor_tensor(
                    ssb[:].rearrange("p (b q) -> p b q", q=P),
                    dlt[:, :, None].to_broadcast([P, B, P]),
                    iot[:, None, :].to_broadcast([P, B, P]), ALU.is_equal)
                sts = gat.tile([P, B * P], bf, tag="sts")
                for ci in range(B):
                    tpp = psB.tile([P, P], bf, tag="pB", name=f"stp{ci}")
                    nc.tensor.transpose(out=tpp[:],
                                        in_=ssb[:, ci * P:(ci + 1) * P],
                                        identity=ids[:])
                    nc.scalar.activation(sts[:, ci * P:(ci + 1) * P], tpp[:],
                                         AF.Copy)
                hg2 = gat.tile([P, B, 10], bf, tag="hg2")
                adp2 = psB.tile([P, B * 2], f32, tag="pAD")
                for ci in range(B):
                    c = c0 + ci
                    t = int(tile_of_chunk[c])
                    if c == int(first_chunk[t]):
                        a2t = sb.tile([P, 2], bf, tag=f"a2t{t % 3}")
                        rows = P if t < NT - 1 else LAST_ROWS
                        if rows < P:
                            nc.vector.memset(a2t[:], 0.0)
                        nc.sync.dma_start(out=a2t[:rows],
                                          in_=adt2[t * P:t * P + rows, :])
                        adt2_of_tile[t] = a2t
                        acc2_of_tile[t] = psA.tile([P, 512], f32, tag="pA",
                                                   name=f"acc2_{t}")
                    nc.gpsimd.indirect_dma_start(
                        out=hg2[:, ci, :], out_offset=None, in_=h2xf[:],
                        in_offset=bass.IndirectOffsetOnAxis(
                            ap=it[:, ci:ci + 1], axis=0))
                    nc.tensor.matmul(out=adp2[:, ci * 2:(ci + 1) * 2],
                                     lhsT=sts[:, ci * P:(ci + 1) * P],
                                     rhs=adt2_of_tile[t][:], start=True,
                                     stop=True)
                t1 = gat.tile([P, B], f32, tag="t1b")
                nc.vector.tensor_tensor(
                    t1[:, :, None], hg2[:, :, 8:9],
                    adp2[:].rearrange("p (b k) -> p b k", k=2)[:, :, 0:1],
                    ALU.mult)
                t2 = gat.tile([P, B], f32, tag="t2b")
                nc.vector.tensor_tensor(
                    t2[:, :, None], hg2[:, :, 9:10],
                    adp2[:].rearrange("p (b k) -> p b k", k=2)[:, :, 1:2],
                    ALU.mult)
                nc.vector.tensor_tensor(
                    hg2[:, :, 8:9], t1[:, :, None], t2[:, :, None], ALU.max)
                nc.vector.tensor_tensor(
                    hg2[:, :, 0:8], hg2[:, :, 0:8],
                    hg2[:, :, 8:9].to_broadcast([P, B, OUT]), ALU.mult)
                for ci in range(B):
                    c = c0 + ci
                    t = int(tile_of_chunk[c])
                    last = (c == int(first_chunk[t]) + int(C_t[t]) - 1)
                    nc.tensor.matmul(out=acc2_of_tile[t][:, 0:9],
                                     lhsT=ssb[:, ci * P:(ci + 1) * P],
                                     rhs=hg2[:, ci, 0:9],
                                     start=(c == int(first_chunk[t])),
                                     stop=last)
                    if last:
                        l2_epilogue(t)

    if not nc.is_finalized():
        nc.finalize()
    return nc


# ---------------------------------------------------------------- runner
_CACHE = {}   # structure key -> (nc, runner)


def _make_runner(nc):
    """Cached-jit replica of bass2jax.run_bass_via_pjrt (axon path)."""
    from concourse.bass2jax import (install_neuronx_cc_hook,
                                    partition_id_tensor, _bass_exec_p)
    install_neuronx_cc_hook()
    partition_name = (nc.partition_id_tensor.name
                      if nc.partition_id_tensor else None)
    in_names, out_names, out_avals = [], [], []
    for alloc in nc.m.functions[0].allocations:
        if not isinstance(alloc, mybir.MemoryLocationSet):
            continue
        name = alloc.memorylocations[0].name
        if alloc.kind == "ExternalInput":
            if name != partition_name:
                in_names.append(name)
        elif alloc.kind == "ExternalOutput":
            out_names.append(name)
            out_avals.append(jax.core.ShapedArray(
                tuple(alloc.tensor_shape), mybir.dt.np(alloc.dtype)))
    n_params = len(in_names)
    n_outs = len(out_avals)
    in_names_all = in_names + out_names + (
        [partition_name] if partition_name else [])
    donate = tuple(range(n_params, n_params + n_outs))

    def _body(*args):
        operands = list(args)
        if partition_name is not None:
            operands.append(partition_id_tensor())
        outs = _bass_exec_p.bind(
            *operands, out_avals=tuple(out_avals),
            in_names=tuple(in_names_all), out_names=tuple(out_names),
            lowering_input_output_aliases=(), sim_require_finite=True,
            sim_require_nnan=True, nc=nc)
        return tuple(outs)

    devices = jax.devices()[:NCORES]
    mesh = Mesh(np.asarray(devices), ("core",))
    sharding = NamedSharding(mesh, PartitionSpec("core"))
    in_specs = (PartitionSpec("core"),) * (n_params + n_outs)
    out_specs = (PartitionSpec("core"),) * len(out_names)
    fn = jax.jit(
        shard_map(_body, mesh=mesh, in_specs=in_specs, out_specs=out_specs,
                  check_rep=False),
        donate_argnums=donate, keep_unused=True)
    return fn, in_names, out_names, out_avals, sharding


_PREV_OUT = [None]   # previous call's device output, donated as the next
                     # zeros-input (the kernel writes every output element)


def kernel(**inputs):
    FG = _prep_feat(inputs)
    # fire the (dominant) feature upload before doing edge bucketing so the
    # tunnel transfer overlaps host prep
    dfeat = dzeros = None
    try:
        devices = jax.devices()[:NCORES]
        mesh = Mesh(np.asarray(devices), ("core",))
        sharding = NamedSharding(mesh, PartitionSpec("core"))
        dfeat = jax.device_put(FG, sharding)
        if _PREV_OUT[0] is not None:
            dzeros = _PREV_OUT[0]
            _PREV_OUT[0] = None
        else:
            dzeros = jax.device_put(
                np.zeros((NCORES * SH, OUT), np.float32), sharding)
    except Exception:
        dfeat = dzeros = None

    blob, C_t, TC, NSUP, toc, fc, Lr = _prep_edges(inputs)
    key = (TC, NSUP, tuple(int(x) for x in C_t))
    if key not in _CACHE:
        nc = _build(C_t, TC, NSUP, toc, fc, Lr)
        _CACHE[key] = (nc, _make_runner(nc))
    nc, (fn, in_names, out_names, out_avals, sharding) = _CACHE[key]

    host_in = {"FEAT": FG, "BLOB": blob.reshape(-1)}
    dev_in = []
    for name in in_names:
        if name == "FEAT" and dfeat is not None:
            dev_in.append(dfeat)
        else:
            dev_in.append(jax.device_put(host_in[name], sharding))
    if dzeros is None:
        dzeros = jax.device_put(
            np.zeros((NCORES * SH, OUT), np.float32), sharding)
    out_arrs = fn(*dev_in, dzeros)
    dev_out = out_arrs[out_names.index("out")]
    out = np.asarray(dev_out)
    _PREV_OUT[0] = dev_out
    return out.reshape(N, OUT)


# revision 19
# speedup vs baseline: 14.2182x; 1.1474x over previous
"""CombinedGAT (2-layer GAT, N=50000, E=800000) on 8 TRN2 NeuronCores.

Strategy (edge parallelism per sharding hint):
- dst-shard nodes across 8 cores (6250 each); each core owns the edges into
  its shard, sorted by dst, padded to a uniform per-dst-tile chunk count so
  one SPMD program serves all cores.
- Phase A is *node-sharded*: core c computes the layer-1 table rows for its
  own 6250 nodes only ([SH, 272] = [h1 (256) | exp(a_src) (8) |
  exp(0.2 a_src) (8)]) using exp(leakyrelu(u+v)) = max(e^u e^v, e^.2u e^.2v),
  then an AllGather replicates the full [N, 272] table. The dst-side exp
  table adt1 [SH, 16] stays local (dst always lands in the owner's shard).
- L1 edge pass: per 128-edge chunk, indirect-DMA gather of h1x rows by src;
  attention weights via gathered exps x St-matmul-expanded dst exps; weighted
  scatter-add into per-dst-tile PSUM via one-hot S matmul (S built on device
  from compact dst-local bytes).
- AllGather of compact layer-2 table [6250,10] -> [50000,10]; L2 edge pass
  identical in structure; log_softmax epilogue.

I/O strategy (the axon tunnel is ~84 MB/s with ~10ms per shard transfer, so
bytes and array count dominate wall time): features are uploaded *sharded*
(2 MB/core instead of 16 MB/core replicated) as one bf16 array, and all
remaining per-core data (edge chunk tables, weights, biases, iota/identity
constants) is packed into ONE int32 blob per core, with bf16 sections read
on device via bitcast APs. Uploads are issued asynchronously so the feature
transfer overlaps the host-side edge bucketing.
"""
import numpy as np
import ml_dtypes

import jax
from jax.sharding import Mesh, NamedSharding, PartitionSpec
from jax.experimental.shard_map import shard_map

import concourse.bass as bass
import concourse.mybir as mybir
import concourse.tile as tile
from concourse import bacc

BF = ml_dtypes.bfloat16
F8 = ml_dtypes.float8_e4m3
P = 128
NCORES = 8
N = 50000
SH = N // NCORES          # 6250 nodes per core
NT = (SH + P - 1) // P    # 49 dst tiles per core
LAST_ROWS = SH - (NT - 1) * P  # 106
HIGH, LOW, EMB = 128, 32, 64
IN1 = HIGH + EMB
HID, HEADS, OUT = 32, 8, 8
IN2 = HID * HEADS
B = 16                    # chunks per super-chunk
NEG = 0.2
FROWS = HIGH + LOW + 1    # feature blob rows per core: highT | lowT | ones

AF = mybir.ActivationFunctionType
ALU = mybir.AluOpType


# ---------------------------------------------------------------- blob layout
def _blob_layout(NSUP):
    """int32 blob: [SRCG u16 | DSTT i32 | DL u8 | bf16 sections], bitcast."""
    S1 = NSUP * P * B            # u16 elements
    S2 = NT * P                  # i32 elements
    secs = {}
    off = S1 // 2 + S2
    n = NSUP * P * B             # DL: u8 elements
    secs["DL"] = (off, n, (NSUP, P, B))
    off += n // 4
    for name, shape in [
        ("W1t", (HIGH, 272)),
        ("W1b", (EMB, 272)),
        ("Wemb", (LOW + 1, EMB)),
        ("W2e", (P, 2, 10)),
        ("idn", (P, P)),
        ("iot", (P, P)),
        ("b1b", (P, IN2)),
        ("b2b", (P, OUT)),
    ]:
        n = int(np.prod(shape))
        assert n % 2 == 0
        secs[name] = (off, n, shape)
        off += n // 2
    return S1, S2, secs, off


# ---------------------------------------------------------------- host prep
_ONES_ROW = np.ones((NCORES, 1, SH), F8)


def _prep_feat(inputs):
    """[8*FROWS, SH] fp8: per core rows = [high^T (128) | low^T (32) | ones].

    fp8-e4m3 features add ~2e-3 rel-fro error on top of the kernel's ~4.5e-3
    (gate 2e-2) and halve the dominant tunnel upload."""
    high = np.asarray(inputs["high_dim_features"], np.float32)
    low = np.asarray(inputs["low_dim_features"], np.float32)
    hp = high.reshape(NCORES, SH, HIGH).transpose(0, 2, 1).astype(F8)
    lp = low.reshape(NCORES, SH, LOW).transpose(0, 2, 1).astype(F8)
    FG = np.concatenate([hp, lp, _ONES_ROW], axis=1)
    return FG.reshape(NCORES * FROWS, SH)


def _prep_edges(inputs):
    """Bucket edges by (dst core, dst tile) into 128-edge chunks; pack blob."""
    ei = np.asarray(inputs["edge_index"])
    loops = np.arange(N, dtype=np.int32)
    src = np.concatenate([ei[0].astype(np.int32), loops])
    dst = np.concatenate([ei[1].astype(np.int32), loops])
    # pack (tile id 9b | src 16b | dst%P 7b) into int32; one radix sort
    # replaces the stable argsort (in-bucket order is irrelevant)
    dlg = dst % SH
    tg0 = ((dst // SH) * NT + dlg // P).astype(np.uint32)
    key = np.sort((tg0 << np.uint32(23))
                  | (src.astype(np.uint32) << np.uint32(7))
                  | (dlg % P).astype(np.uint32))
    tg = (key >> np.uint32(23)).astype(np.int64)
    ss = ((key >> np.uint32(7)) & np.uint32(0xFFFF)).astype(np.int32)
    dlp = (key & np.uint32(0x7F)).astype(np.int32)
    starts = np.searchsorted(tg, np.arange(NCORES * NT + 1))
    cnt = np.diff(starts).reshape(NCORES, NT)
    C_t = np.maximum(1, np.ceil(cnt.max(axis=0) / P).astype(np.int64))
    TC = int(C_t.sum())
    NSUP = (TC + B - 1) // B
    # pad the last tile's chunk range to the full NSUP*B slot count so the
    # device loop is uniform (pad chunks have dl=-1 -> zero one-hot)
    C_t[NT - 1] += NSUP * B - TC
    TC = NSUP * B
    first_chunk = np.concatenate([[0], np.cumsum(C_t)])[:NT]
    tile_of_chunk = np.repeat(np.arange(NT), C_t)

    pos = np.arange(len(tg)) - starts[tg]
    core = tg // NT
    tloc = tg - core * NT
    flat = (core.astype(np.int64) * TC + first_chunk[tloc] + pos // P) * P \
        + pos % P
    srcg = np.zeros((NCORES, TC, P), np.uint16)
    dlc = np.full((NCORES, TC, P), 255, np.uint8)   # 255 = no-edge sentinel
    srcg.reshape(-1)[flat] = ss.astype(np.uint16)
    dlc.reshape(-1)[flat] = dlp.astype(np.uint8)
    srcg_dev = np.ascontiguousarray(
        srcg.reshape(NCORES, NSUP, B, P).transpose(0, 1, 3, 2))  # [8,NSUP,P,B]
    dl_dev = np.ascontiguousarray(
        dlc.reshape(NCORES, NSUP, B, P).transpose(0, 1, 3, 2))

    S1, S2, secs, Lr = _blob_layout(NSUP)
    blob = np.empty((NCORES, Lr), np.int32)
    blob[:, :S1 // 2] = srcg_dev.reshape(NCORES, S1).view(np.int32)
    dstt = np.minimum(np.arange(NT)[:, None] * P + np.arange(P)[None, :],
                      SH - 1).astype(np.int32)                   # local ids
    blob[:, S1 // 2:S1 // 2 + S2] = dstt.reshape(-1)[None, :]
    off, n, _ = secs["DL"]
    blob[:, off:off + n // 4] = dl_dev.reshape(NCORES, n).view(np.int32)

    # weight folding (replicated across cores)
    W1 = np.asarray(inputs["W1"], np.float32)          # [192, 256]
    as1 = np.asarray(inputs["att_src1"], np.float32)
    ad1 = np.asarray(inputs["att_dst1"], np.float32)
    W1As = np.einsum("khj,hj->kh", W1.reshape(IN1, HEADS, HID), as1)
    W1Ad = np.einsum("khj,hj->kh", W1.reshape(IN1, HEADS, HID), ad1)
    W1ex = np.concatenate([W1, W1As, W1Ad], axis=1).astype(BF)   # [192, 272]
    W2 = np.asarray(inputs["W2"], np.float32)          # [256, 8]
    W2As = W2 @ np.asarray(inputs["att_src2"], np.float32).reshape(OUT, 1)
    W2Ad = W2 @ np.asarray(inputs["att_dst2"], np.float32).reshape(OUT, 1)
    W2ex = np.concatenate([W2, W2As, W2Ad], axis=1).astype(BF)   # [256, 10]
    Wemb = np.asarray(inputs["W_emb"], np.float32)
    Wemb1 = np.concatenate(
        [Wemb, np.asarray(inputs["b_emb"], np.float32)[None, :]],
        axis=0).astype(BF)                             # [33, 64]
    wparts = {
        "W1t": np.ascontiguousarray(W1ex[:HIGH]),
        "W1b": np.ascontiguousarray(W1ex[HIGH:]),
        "Wemb": Wemb1,
        "W2e": np.ascontiguousarray(
            W2ex.reshape(2, P, 10).transpose(1, 0, 2)),          # [P, 2, 10]
        "idn": np.eye(P, dtype=np.float32).astype(BF),
        "iot": np.broadcast_to(np.arange(P, dtype=np.float32),
                               (P, P)).astype(BF).copy(),
        "b1b": np.broadcast_to(np.asarray(inputs["b1"], np.float32),
                               (P, IN2)).astype(BF).copy(),
        "b2b": np.broadcast_to(np.asarray(inputs["b2"], np.float32),
                               (P, OUT)).astype(BF).copy(),
    }
    for name, arr in wparts.items():
        off, n, shape = secs[name]
        assert arr.shape == shape, (name, arr.shape, shape)
        blob[:, off:off + n // 2] = arr.reshape(-1).view(np.int32)[None, :]

    return blob, C_t, TC, NSUP, tile_of_chunk, first_chunk, Lr


# ---------------------------------------------------------------- device build
def _build(C_t, TC, NSUP, tile_of_chunk, first_chunk, Lr):
    nc = bacc.Bacc("TRN2", target_bir_lowering=False, debug=False,
                   num_devices=NCORES)
    bf, f32, i32 = mybir.dt.bfloat16, mybir.dt.float32, mybir.dt.int32

    f8 = mybir.dt.float8e4
    u16, u8 = mybir.dt.uint16, mybir.dt.uint8
    FEAT = nc.dram_tensor("FEAT", [FROWS, SH], f8, kind="ExternalInput")
    BLOB = nc.dram_tensor("BLOB", [Lr], i32, kind="ExternalInput")
    out_d = nc.dram_tensor("out", [SH, OUT], bf, kind="ExternalOutput")

    adt1 = nc.dram_tensor("adt1", [SH, 16], bf)
    adt2 = nc.dram_tensor("adt2", [SH, 2], bf)

    S1, S2, secs, Lr2 = _blob_layout(NSUP)
    assert Lr2 == Lr

    def sec_ap(name):
        off, n, shape = secs[name]
        ap = BLOB[off:off + n // 2].bitcast(bf)
        if len(shape) == 2:
            return ap.rearrange("(a b) -> a b", b=shape[1])
        return ap.rearrange("(a b c) -> a b c", b=shape[1], c=shape[2])

    def srcg_ap(s):
        return BLOB[s * P * B // 2:(s + 1) * P * B // 2] \
            .bitcast(u16).rearrange("(p b) -> p b", b=B)

    def dl_ap(s):
        off = secs["DL"][0]
        return BLOB[off + s * P * B // 4:off + (s + 1) * P * B // 4] \
            .bitcast(u8).rearrange("(p b) -> p b", b=B)

    def dstt_ap(t):
        base = S1 // 2 + t * P
        return BLOB[base:base + P].rearrange("(p a) -> p a", a=1)

    with tile.TileContext(nc) as tc:
        with tc.tile_pool(name="const", bufs=1) as cpool, \
             tc.tile_pool(name="sb", bufs=3) as sb, \
             tc.tile_pool(name="gat", bufs=3) as gat, \
             tc.tile_pool(name="psA", bufs=2, space="PSUM") as psA, \
             tc.tile_pool(name="psB", bufs=3, space="PSUM") as psB, \
             tc.tile_pool(name="dram", bufs=1, space="DRAM") as dram:

            h1l = dram.tile([SH, 272], bf)
            h1x = dram.tile([N, 272], bf)

            w1t = cpool.tile([HIGH, 272], bf)
            nc.sync.dma_start(out=w1t[:], in_=sec_ap("W1t"))
            w1b = cpool.tile([EMB, 272], bf)
            nc.sync.dma_start(out=w1b[:], in_=sec_ap("W1b"))
            wem = cpool.tile([LOW + 1, EMB], bf)
            nc.sync.dma_start(out=wem[:], in_=sec_ap("Wemb"))
            w2e = cpool.tile([P, 2, 10], bf)
            nc.sync.dma_start(out=w2e[:], in_=sec_ap("W2e"))
            b1s = cpool.tile([P, IN2], bf)
            nc.sync.dma_start(out=b1s[:], in_=sec_ap("b1b"))
            b2s = cpool.tile([P, OUT], bf)
            nc.sync.dma_start(out=b2s[:], in_=sec_ap("b2b"))
            ids = cpool.tile([P, P], bf)
            nc.sync.dma_start(out=ids[:], in_=sec_ap("idn"))
            iot = cpool.tile([P, P], bf)
            nc.sync.dma_start(out=iot[:], in_=sec_ap("iot"))

            # -------- Phase A: layer-1 tables for this core's SH nodes -------
            for ntile in range(NT):
                n0 = ntile * P
                w = min(P, SH - n0)
                ht8 = sb.tile([P, P], f8, tag="ht8")
                nc.sync.dma_start(out=ht8[:, :w], in_=FEAT[0:HIGH, n0:n0 + w])
                lt8 = sb.tile([LOW + 1, P], f8, tag="lt8")
                nc.sync.dma_start(out=lt8[:, :w], in_=FEAT[HIGH:FROWS, n0:n0 + w])
                ht = sb.tile([P, P], bf, tag="ht")
                nc.scalar.activation(ht[:, :w], ht8[:, :w], AF.Copy)
                lt = sb.tile([LOW + 1, P], bf, tag="lt")
                nc.scalar.activation(lt[:, :w], lt8[:, :w], AF.Copy)
                embp = psB.tile([EMB, P], f32, tag="pB")
                nc.tensor.matmul(out=embp[:, :w], lhsT=wem[:], rhs=lt[:, :w],
                                 start=True, stop=True)
                # elu(v) = max(v,0)-1 + exp(-relu(-v))
                tm = sb.tile([EMB, P], f32, tag="tm")
                nc.scalar.activation(tm[:, :w], embp[:, :w], AF.Relu, scale=-1.0)
                te = sb.tile([EMB, P], f32, tag="te")
                nc.scalar.activation(te[:, :w], tm[:, :w], AF.Exp, scale=-1.0)
                tr = sb.tile([EMB, P], f32, tag="tr")
                nc.vector.tensor_scalar(tr[:, :w], embp[:, :w], 0.0, -1.0,
                                        ALU.max, ALU.add)
                embs = sb.tile([EMB, P], bf, tag="embs")
                nc.vector.tensor_tensor(embs[:, :w], tr[:, :w], te[:, :w],
                                        ALU.add)
                h1p = psA.tile([P, 512], f32, tag="pA")
                nc.tensor.matmul(out=h1p[:w, 0:272], lhsT=ht[:, :w], rhs=w1t[:],
                                 start=True, stop=False)
                nc.tensor.matmul(out=h1p[:w, 0:272], lhsT=embs[:, :w],
                                 rhs=w1b[:], start=False, stop=True)
                h1s = sb.tile([P, 272], bf, tag="h1s")
                nc.scalar.activation(h1s[:w, 0:256], h1p[:w, 0:256], AF.Copy)
                ads = sb.tile([P, 16], bf, tag="ads")
                nc.scalar.activation(h1s[:w, 256:264], h1p[:w, 256:264], AF.Exp)
                nc.scalar.activation(h1s[:w, 264:272], h1p[:w, 256:264], AF.Exp,
                                     scale=NEG)
                nc.scalar.activation(ads[:w, 0:8], h1p[:w, 264:272], AF.Exp)
                nc.scalar.activation(ads[:w, 8:16], h1p[:w, 264:272], AF.Exp,
                                     scale=NEG)
                nc.sync.dma_start(out=h1l[n0:n0 + w, :], in_=h1s[:w])
                nc.sync.dma_start(out=adt1[n0:n0 + w, :], in_=ads[:w])

            # -------- replicate the layer-1 table --------
            nc.gpsimd.collective_compute(
                "AllGather", ALU.bypass,
                replica_groups=[list(range(NCORES))],
                ins=[h1l.opt()], outs=[h1x.opt()])

            # ---------------- L1 edge pass ----------------
            h2xl = dram.tile([SH, 10], bf)
            h2xf = dram.tile([N, 10], bf)

            acc_of_tile = {}
            adt_of_tile = {}

            def l1_epilogue(t):
                rows = P if t < NT - 1 else LAST_ROWS
                acc = acc_of_tile.pop(t)
                rz = sb.tile([P, 8], f32, tag="rz")
                nc.vector.reciprocal(rz[:rows], acc[:rows, 256:264])
                xr = sb.tile([P, IN2], f32, tag="xr")
                nc.vector.tensor_tensor(
                    xr[:rows],
                    acc[:rows, 0:256].rearrange("p (h j) -> p h j", j=HID),
                    rz[:rows, :, None].to_broadcast([rows, 8, HID]), ALU.mult)
                nc.vector.tensor_tensor(xr[:rows], xr[:rows], b1s[:rows],
                                        ALU.add)
                tm = sb.tile([P, IN2], f32, tag="etm")
                nc.scalar.activation(tm[:rows], xr[:rows], AF.Relu, scale=-1.0)
                te = sb.tile([P, IN2], f32, tag="ete")
                nc.scalar.activation(te[:rows], tm[:rows], AF.Exp, scale=-1.0)
                tr = sb.tile([P, IN2], f32, tag="etr")
                nc.vector.tensor_scalar(tr[:rows], xr[:rows], 0.0, -1.0,
                                        ALU.max, ALU.add)
                x2 = sb.tile([P, IN2], bf, tag="x2")
                if rows < P:
                    nc.vector.memset(x2[:], 0.0)
                nc.vector.tensor_tensor(x2[:rows], tr[:rows], te[:rows],
                                        ALU.add)
                # x2T blocks + h2x row
                x2tb = sb.tile([P, 2, P], bf, tag="x2tb")
                for k in range(2):
                    tp = psB.tile([P, P], bf, tag="pB")
                    nc.tensor.transpose(out=tp[:], in_=x2[:, k * P:(k + 1) * P],
                                        identity=ids[:])
                    nc.vector.tensor_copy(x2tb[:, k, :], tp[:])
                h2p = psB.tile([P, 16], f32, tag="pB")
                for k in range(2):
                    nc.tensor.matmul(out=h2p[:, 0:10], lhsT=x2tb[:, k, :],
                                     rhs=w2e[:, k, :], start=(k == 0),
                                     stop=(k == 1))
                h2r = sb.tile([P, 10], bf, tag="h2r")
                nc.scalar.activation(h2r[:rows, 0:8], h2p[:rows, 0:8], AF.Copy)
                nc.scalar.activation(h2r[:rows, 8:9], h2p[:rows, 8:9], AF.Exp)
                nc.scalar.activation(h2r[:rows, 9:10], h2p[:rows, 8:9], AF.Exp,
                                     scale=NEG)
                a2r = sb.tile([P, 2], bf, tag="a2r")
                nc.scalar.activation(a2r[:rows, 0:1], h2p[:rows, 9:10], AF.Exp)
                nc.scalar.activation(a2r[:rows, 1:2], h2p[:rows, 9:10], AF.Exp,
                                     scale=NEG)
                nc.sync.dma_start(out=h2xl[t * P:t * P + rows, :], in_=h2r[:rows])
                nc.sync.dma_start(out=adt2[t * P:t * P + rows, :], in_=a2r[:rows])

            for s in range(NSUP):
                c0 = s * B
                it16 = gat.tile([P, B], u16, tag="it16")
                nc.sync.dma_start(out=it16[:], in_=srcg_ap(s))
                it = gat.tile([P, B], i32, tag="it")
                nc.vector.tensor_copy(it[:], it16[:])
                dlt8 = gat.tile([P, B], u8, tag="dlt8")
                nc.sync.dma_start(out=dlt8[:], in_=dl_ap(s))
                dlt = gat.tile([P, B], bf, tag="dlt")
                nc.vector.tensor_copy(dlt[:], dlt8[:])
                ssb = gat.tile([P, B * P], bf, tag="ssb")
                nc.vector.tensor_tensor(
                    ssb[:].rearrange("p (b q) -> p b q", q=P),
                    dlt[:, :, None].to_broadcast([P, B, P]),
                    iot[:, None, :].to_broadcast([P, B, P]), ALU.is_equal)
                sts = gat.tile([P, B * P], bf, tag="sts")
                for ci in range(B):
                    tpp = psB.tile([P, P], bf, tag="pB", name=f"stp{ci}")
                    nc.tensor.transpose(out=tpp[:],
                                        in_=ssb[:, ci * P:(ci + 1) * P],
                                        identity=ids[:])
                    nc.scalar.activation(sts[:, ci * P:(ci + 1) * P], tpp[:],
                                         AF.Copy)
                hg = gat.tile([P, B, 272], bf, tag="hg")
                adp = psB.tile([P, B * 16], f32, tag="pAD")
                for ci in range(B):
                    c = c0 + ci
                    t = int(tile_of_chunk[c])
                    if c == int(first_chunk[t]):
                        dtt = sb.tile([P, 1], i32, tag="dtt")
                        nc.sync.dma_start(out=dtt[:], in_=dstt_ap(t))
                        adtt = sb.tile([P, 16], bf, tag=f"adtt{t % 3}")
                        nc.gpsimd.indirect_dma_start(
                            out=adtt[:], out_offset=None, in_=adt1[:],
                            in_offset=bass.IndirectOffsetOnAxis(
                                ap=dtt[:, :1], axis=0))
                        adt_of_tile[t] = adtt
                        acc_of_tile[t] = psA.tile([P, 512], f32, tag="pA",
                                                  name=f"acc{t}")
                    nc.gpsimd.indirect_dma_start(
                        out=hg[:, ci, :], out_offset=None, in_=h1x[:],
                        in_offset=bass.IndirectOffsetOnAxis(
                            ap=it[:, ci:ci + 1], axis=0))
                    nc.tensor.matmul(out=adp[:, ci * 16:(ci + 1) * 16],
                                     lhsT=sts[:, ci * P:(ci + 1) * P],
                                     rhs=adt_of_tile[t][:], start=True,
                                     stop=True)
                # batched attention weights
                t1 = gat.tile([P, B * 8], f32, tag="t1")
                nc.vector.tensor_tensor(
                    t1[:].rearrange("p (b h) -> p b h", h=8),
                    hg[:, :, 256:264],
                    adp[:].rearrange("p (b h) -> p b h", h=16)[:, :, 0:8],
                    ALU.mult)
                t2 = gat.tile([P, B * 8], f32, tag="t2")
                nc.vector.tensor_tensor(
                    t2[:].rearrange("p (b h) -> p b h", h=8),
                    hg[:, :, 264:272],
                    adp[:].rearrange("p (b h) -> p b h", h=16)[:, :, 8:16],
                    ALU.mult)
                nc.vector.tensor_tensor(
                    hg[:, :, 256:264],
                    t1[:].rearrange("p (b h) -> p b h", h=8),
                    t2[:].rearrange("p (b h) -> p b h", h=8),
                    ALU.max)
                nc.vector.tensor_tensor(
                    hg[:, :, 0:256].rearrange("p b (h j) -> p b h j", j=HID),
                    hg[:, :, 0:256].rearrange("p b (h j) -> p b h j", j=HID),
                    hg[:, :, 256:264][:, :, :, None].to_broadcast(
                        [P, B, 8, HID]),
                    ALU.mult)
                for ci in range(B):
                    c = c0 + ci
                    t = int(tile_of_chunk[c])
                    last = (c == int(first_chunk[t]) + int(C_t[t]) - 1)
                    nc.tensor.matmul(out=acc_of_tile[t][:, 0:264],
                                     lhsT=ssb[:, ci * P:(ci + 1) * P],
                                     rhs=hg[:, ci, 0:264],
                                     start=(c == int(first_chunk[t])),
                                     stop=last)
                    if last:
                        l1_epilogue(t)

            # ---------------- AllGather layer-2 table ----------------
            nc.gpsimd.collective_compute(
                "AllGather", ALU.bypass,
                replica_groups=[list(range(NCORES))],
                ins=[h2xl.opt()], outs=[h2xf.opt()])

            # ---------------- L2 edge pass ----------------
            acc2_of_tile = {}
            adt2_of_tile = {}

            def l2_epilogue(t):
                rows = P if t < NT - 1 else LAST_ROWS
                acc = acc2_of_tile.pop(t)
                rz = sb.tile([P, 1], f32, tag="rz2")
                nc.vector.reciprocal(rz[:rows], acc[:rows, 8:9])
                o = sb.tile([P, OUT], f32, tag="o2")
                nc.vector.tensor_tensor(
                    o[:rows], acc[:rows, 0:8],
                    rz[:rows, :].to_broadcast([rows, OUT]), ALU.mult)
                nc.vector.tensor_tensor(o[:rows], o[:rows], b2s[:rows], ALU.add)
                ex = sb.tile([P, OUT], f32, tag="ex2")
                nc.scalar.activation(ex[:rows], o[:rows], AF.Exp)
                sm = sb.tile([P, 1], f32, tag="sm2")
                nc.vector.reduce_sum(sm[:rows], ex[:rows],
                                     axis=mybir.AxisListType.X)
                lg = sb.tile([P, 1], f32, tag="lg2")
                nc.scalar.activation(lg[:rows], sm[:rows], AF.Ln)
                fo = sb.tile([P, OUT], bf, tag="fo2")
                nc.vector.tensor_tensor(
                    fo[:rows], o[:rows],
                    lg[:rows, :].to_broadcast([rows, OUT]), ALU.subtract)
                nc.sync.dma_start(out=out_d[t * P:t * P + rows, :], in_=fo[:rows])

            for s in range(NSUP):
                c0 = s * B
                it16 = gat.tile([P, B], u16, tag="it16")
                nc.sync.dma_start(out=it16[:], in_=srcg_ap(s))
                it = gat.tile([P, B], i32, tag="it")
                nc.vector.tensor_copy(it[:], it16[:])
                dlt8 = gat.tile([P, B], u8, tag="dlt8")
                nc.sync.dma_start(out=dlt8[:], in_=dl_ap(s))
                dlt = gat.tile([P, B], bf, tag="dlt")
                nc.vector.tensor_copy(dlt[:], dlt8[:])
                ssb = gat.tile([P, B * P], bf, tag="ssb")
                nc.vector.tensor_tensor(
                    ssb[:].rearrange("p (b q) -> p b q", q=P),
                    dlt[:, :, None].to_broadcast([P, B, P]),
                    iot[:, None, :].to_broadcast([P, B, P]), ALU.is_equal)
                sts = gat.tile([P, B * P], bf, tag="sts")
                for ci in range(B):
                    tpp = psB.tile([P, P], bf, tag="pB", name=f"stp{ci}")
                    nc.tensor.transpose(out=tpp[:],
                                        in_=ssb[:, ci * P:(ci + 1) * P],
                                        identity=ids[:])
                    nc.scalar.activation(sts[:, ci * P:(ci + 1) * P], tpp[:],
                                         AF.Copy)
                hg2 = gat.tile([P, B, 10], bf, tag="hg2")
                adp2 = psB.tile([P, B * 2], f32, tag="pAD")
                for ci in range(B):
                    c = c0 + ci
                    t = int(tile_of_chunk[c])
                    if c == int(first_chunk[t]):
                        a2t = sb.tile([P, 2], bf, tag=f"a2t{t % 3}")
                        rows = P if t < NT - 1 else LAST_ROWS
                        if rows < P:
                            nc.vector.memset(a2t[:], 0.0)
                        nc.sync.dma_start(out=a2t[:rows],
                                          in_=adt2[t * P:t * P + rows, :])
                        adt2_of_tile[t] = a2t
                        acc2_of_tile[t] = psA.tile([P, 512], f32, tag="pA",
                                                   name=f"acc2_{t}")
                    nc.gpsimd.indirect_dma_start(
                        out=hg2[:, ci, :], out_offset=None, in_=h2xf[:],
                        in_offset=bass.IndirectOffsetOnAxis(
                            ap=it[:, ci:ci + 1], axis=0))
                    nc.tensor.matmul(out=adp2[:, ci * 2:(ci + 1) * 2],
                                     lhsT=sts[:, ci * P:(ci + 1) * P],
                                     rhs=adt2_of_tile[t][:], start=True,
                                     stop=True)
                t1 = gat.tile([P, B], f32, tag="t1b")
                nc.vector.tensor_tensor(
                    t1[:, :, None], hg2[:, :, 8:9],
                    adp2[:].rearrange("p (b k) -> p b k", k=2)[:, :, 0:1],
                    ALU.mult)
                t2 = gat.tile([P, B], f32, tag="t2b")
                nc.vector.tensor_tensor(
                    t2[:, :, None], hg2[:, :, 9:10],
                    adp2[:].rearrange("p (b k) -> p b k", k=2)[:, :, 1:2],
                    ALU.mult)
                nc.vector.tensor_tensor(
                    hg2[:, :, 8:9], t1[:, :, None], t2[:, :, None], ALU.max)
                nc.vector.tensor_tensor(
                    hg2[:, :, 0:8], hg2[:, :, 0:8],
                    hg2[:, :, 8:9].to_broadcast([P, B, OUT]), ALU.mult)
                for ci in range(B):
                    c = c0 + ci
                    t = int(tile_of_chunk[c])
                    last = (c == int(first_chunk[t]) + int(C_t[t]) - 1)
                    nc.tensor.matmul(out=acc2_of_tile[t][:, 0:9],
                                     lhsT=ssb[:, ci * P:(ci + 1) * P],
                                     rhs=hg2[:, ci, 0:9],
                                     start=(c == int(first_chunk[t])),
                                     stop=last)
                    if last:
                        l2_epilogue(t)

    if not nc.is_finalized():
        nc.finalize()
    return nc


# ---------------------------------------------------------------- runner
_CACHE = {}   # structure key -> (nc, runner)


def _make_runner(nc):
    """Cached-jit replica of bass2jax.run_bass_via_pjrt (axon path)."""
    from concourse.bass2jax import (install_neuronx_cc_hook,
                                    partition_id_tensor, _bass_exec_p)
    install_neuronx_cc_hook()
    partition_name = (nc.partition_id_tensor.name
                      if nc.partition_id_tensor else None)
    in_names, out_names, out_avals = [], [], []
    for alloc in nc.m.functions[0].allocations:
        if not isinstance(alloc, mybir.MemoryLocationSet):
            continue
        name = alloc.memorylocations[0].name
        if alloc.kind == "ExternalInput":
            if name != partition_name:
                in_names.append(name)
        elif alloc.kind == "ExternalOutput":
            out_names.append(name)
            out_avals.append(jax.core.ShapedArray(
                tuple(alloc.tensor_shape), mybir.dt.np(alloc.dtype)))
    n_params = len(in_names)
    n_outs = len(out_avals)
    in_names_all = in_names + out_names + (
        [partition_name] if partition_name else [])
    donate = tuple(range(n_params, n_params + n_outs))

    def _body(*args):
        operands = list(args)
        if partition_name is not None:
            operands.append(partition_id_tensor())
        outs = _bass_exec_p.bind(
            *operands, out_avals=tuple(out_avals),
            in_names=tuple(in_names_all), out_names=tuple(out_names),
            lowering_input_output_aliases=(), sim_require_finite=True,
            sim_require_nnan=True, nc=nc)
        return tuple(outs)

    devices = jax.devices()[:NCORES]
    mesh = Mesh(np.asarray(devices), ("core",))
    sharding = NamedSharding(mesh, PartitionSpec("core"))
    in_specs = (PartitionSpec("core"),) * (n_params + n_outs)
    out_specs = (PartitionSpec("core"),) * len(out_names)
    fn = jax.jit(
        shard_map(_body, mesh=mesh, in_specs=in_specs, out_specs=out_specs,
                  check_rep=False),
        donate_argnums=donate, keep_unused=True)
    return fn, in_names, out_names, out_avals, sharding


_PREV_OUT = [None]   # previous call's device output, donated as the next
                     # zeros-input (the kernel writes every output element)


def kernel(**inputs):
    FG = _prep_feat(inputs)
    # fire the (dominant) feature upload before doing edge bucketing so the
    # tunnel transfer overlaps host prep
    dfeat = dzeros = None
    try:
        devices = jax.devices()[:NCORES]
        mesh = Mesh(np.asarray(devices), ("core",))
        sharding = NamedSharding(mesh, PartitionSpec("core"))
        dfeat = jax.device_put(FG, sharding)
        if _PREV_OUT[0] is not None:
            dzeros = _PREV_OUT[0]
            _PREV_OUT[0] = None
        else:
            dzeros = jax.device_put(
                np.zeros((NCORES * SH, OUT), BF), sharding)
    except Exception:
        dfeat = dzeros = None

    blob, C_t, TC, NSUP, toc, fc, Lr = _prep_edges(inputs)
    key = (TC, NSUP, tuple(int(x) for x in C_t))
    if key not in _CACHE:
        nc = _build(C_t, TC, NSUP, toc, fc, Lr)
        _CACHE[key] = (nc, _make_runner(nc))
    nc, (fn, in_names, out_names, out_avals, sharding) = _CACHE[key]

    host_in = {"FEAT": FG, "BLOB": blob.reshape(-1)}
    dev_in = []
    for name in in_names:
        if name == "FEAT" and dfeat is not None:
            dev_in.append(dfeat)
        else:
            dev_in.append(jax.device_put(host_in[name], sharding))
    if dzeros is None:
        dzeros = jax.device_put(
            np.zeros((NCORES * SH, OUT), BF), sharding)
    out_arrs = fn(*dev_in, dzeros)
    dev_out = out_arrs[out_names.index("out")]
    out = np.asarray(dev_out).astype(np.float32)
    _PREV_OUT[0] = dev_out
    return out.reshape(N, OUT)


# revision 20
# speedup vs baseline: 15.4916x; 1.0896x over previous
"""CombinedGAT (2-layer GAT, N=50000, E=800000) on 8 TRN2 NeuronCores.

Strategy (edge parallelism per sharding hint):
- dst-shard nodes across 8 cores (6250 each); each core owns the edges into
  its shard, sorted by dst, padded to a uniform per-dst-tile chunk count so
  one SPMD program serves all cores.
- Phase A is *node-sharded*: core c computes the layer-1 table rows for its
  own 6250 nodes only ([SH, 272] = [h1 (256) | exp(a_src) (8) |
  exp(0.2 a_src) (8)]) using exp(leakyrelu(u+v)) = max(e^u e^v, e^.2u e^.2v),
  then an AllGather replicates the full [N, 272] table. The dst-side exp
  table adt1 [SH, 16] stays local (dst always lands in the owner's shard).
- L1 edge pass: per 128-edge chunk, indirect-DMA gather of h1x rows by src;
  attention weights via gathered exps x St-matmul-expanded dst exps; weighted
  scatter-add into per-dst-tile PSUM via one-hot S matmul (S built on device
  from compact dst-local bytes).
- AllGather of compact layer-2 table [6250,10] -> [50000,10]; L2 edge pass
  identical in structure; log_softmax epilogue.

I/O strategy (the axon tunnel is ~84 MB/s with ~10ms per shard transfer, so
bytes and array count dominate wall time): features are uploaded *sharded*
(2 MB/core instead of 16 MB/core replicated) as one bf16 array, and all
remaining per-core data (edge chunk tables, weights, biases, iota/identity
constants) is packed into ONE int32 blob per core, with bf16 sections read
on device via bitcast APs. Uploads are issued asynchronously so the feature
transfer overlaps the host-side edge bucketing.
"""
import numpy as np
import ml_dtypes

import jax
from jax.sharding import Mesh, NamedSharding, PartitionSpec
from jax.experimental.shard_map import shard_map

import concourse.bass as bass
import concourse.mybir as mybir
import concourse.tile as tile
from concourse import bacc

BF = ml_dtypes.bfloat16
F8 = ml_dtypes.float8_e4m3
P = 128
NCORES = 8
N = 50000
SH = N // NCORES          # 6250 nodes per core
NT = (SH + P - 1) // P    # 49 dst tiles per core
LAST_ROWS = SH - (NT - 1) * P  # 106
HIGH, LOW, EMB = 128, 32, 64
IN1 = HIGH + EMB
HID, HEADS, OUT = 32, 8, 8
IN2 = HID * HEADS
B = 16                    # chunks per super-chunk
NEG = 0.2
FROWS = HIGH + LOW + 1    # feature blob rows per core: highT | lowT | ones

AF = mybir.ActivationFunctionType
ALU = mybir.AluOpType


# ---------------------------------------------------------------- blob layout
def _blob_layout(NSUP):
    """int32 blob: [SRCG u16 | DSTT i32 | DL u8 | bf16 sections], bitcast."""
    S1 = NSUP * P * B            # u16 elements
    S2 = NT * P                  # i32 elements
    secs = {}
    off = S1 // 2 + S2
    n = NSUP * P * B             # DL: u8 elements
    secs["DL"] = (off, n, (NSUP, P, B))
    off += n // 4
    for name, shape in [
        ("W1t", (HIGH, 272)),
        ("W1b", (EMB, 272)),
        ("Wemb", (LOW + 1, EMB)),
        ("W2e", (P, 2, 10)),
        ("idn", (P, P)),
        ("iot", (P, P)),
        ("b1b", (P, IN2)),
        ("b2b", (P, OUT)),
    ]:
        n = int(np.prod(shape))
        assert n % 2 == 0
        secs[name] = (off, n, shape)
        off += n // 2
    return S1, S2, secs, off


# ---------------------------------------------------------------- host prep
_ONES_ROW = np.ones((NCORES, 1, SH), F8)


def _prep_feat(inputs):
    """[8*FROWS, SH] fp8: per core rows = [high^T (128) | low^T (32) | ones].

    fp8-e4m3 features add ~2e-3 rel-fro error on top of the kernel's ~4.5e-3
    (gate 2e-2) and halve the dominant tunnel upload."""
    high = np.asarray(inputs["high_dim_features"], np.float32)
    low = np.asarray(inputs["low_dim_features"], np.float32)
    hp = high.reshape(NCORES, SH, HIGH).transpose(0, 2, 1).astype(F8)
    lp = low.reshape(NCORES, SH, LOW).transpose(0, 2, 1).astype(F8)
    FG = np.concatenate([hp, lp, _ONES_ROW], axis=1)
    return FG.reshape(NCORES * FROWS, SH)


def _prep_edges(inputs):
    """Bucket edges by (dst core, dst tile) into 128-edge chunks; pack blob."""
    ei = np.asarray(inputs["edge_index"])
    loops = np.arange(N, dtype=np.int32)
    src = np.concatenate([ei[0].astype(np.int32), loops])
    dst = np.concatenate([ei[1].astype(np.int32), loops])
    # pack (tile id 9b | src 16b | dst%P 7b) into int32; one radix sort
    # replaces the stable argsort (in-bucket order is irrelevant)
    dlg = dst % SH
    tg0 = ((dst // SH) * NT + dlg // P).astype(np.uint32)
    key = np.sort((tg0 << np.uint32(23))
                  | (src.astype(np.uint32) << np.uint32(7))
                  | (dlg % P).astype(np.uint32))
    tg = (key >> np.uint32(23)).astype(np.int64)
    ss = ((key >> np.uint32(7)) & np.uint32(0xFFFF)).astype(np.int32)
    dlp = (key & np.uint32(0x7F)).astype(np.int32)
    starts = np.searchsorted(tg, np.arange(NCORES * NT + 1))
    cnt = np.diff(starts).reshape(NCORES, NT)
    C_t = np.maximum(1, np.ceil(cnt.max(axis=0) / P).astype(np.int64))
    TC = int(C_t.sum())
    NSUP = (TC + B - 1) // B
    # pad the last tile's chunk range to the full NSUP*B slot count so the
    # device loop is uniform (pad chunks have dl=-1 -> zero one-hot)
    C_t[NT - 1] += NSUP * B - TC
    TC = NSUP * B
    first_chunk = np.concatenate([[0], np.cumsum(C_t)])[:NT]
    tile_of_chunk = np.repeat(np.arange(NT), C_t)

    pos = np.arange(len(tg)) - starts[tg]
    core = tg // NT
    tloc = tg - core * NT
    flat = (core.astype(np.int64) * TC + first_chunk[tloc] + pos // P) * P \
        + pos % P
    srcg = np.zeros((NCORES, TC, P), np.uint16)
    dlc = np.full((NCORES, TC, P), 255, np.uint8)   # 255 = no-edge sentinel
    srcg.reshape(-1)[flat] = ss.astype(np.uint16)
    dlc.reshape(-1)[flat] = dlp.astype(np.uint8)
    srcg_dev = np.ascontiguousarray(
        srcg.reshape(NCORES, NSUP, B, P).transpose(0, 1, 3, 2))  # [8,NSUP,P,B]
    dl_dev = np.ascontiguousarray(
        dlc.reshape(NCORES, NSUP, B, P).transpose(0, 1, 3, 2))

    S1, S2, secs, Lr = _blob_layout(NSUP)
    blob = np.empty((NCORES, Lr), np.int32)
    blob[:, :S1 // 2] = srcg_dev.reshape(NCORES, S1).view(np.int32)
    dstt = np.minimum(np.arange(NT)[:, None] * P + np.arange(P)[None, :],
                      SH - 1).astype(np.int32)                   # local ids
    blob[:, S1 // 2:S1 // 2 + S2] = dstt.reshape(-1)[None, :]
    off, n, _ = secs["DL"]
    blob[:, off:off + n // 4] = dl_dev.reshape(NCORES, n).view(np.int32)

    # weight folding (replicated across cores)
    W1 = np.asarray(inputs["W1"], np.float32)          # [192, 256]
    as1 = np.asarray(inputs["att_src1"], np.float32)
    ad1 = np.asarray(inputs["att_dst1"], np.float32)
    W1As = np.einsum("khj,hj->kh", W1.reshape(IN1, HEADS, HID), as1)
    W1Ad = np.einsum("khj,hj->kh", W1.reshape(IN1, HEADS, HID), ad1)
    W1ex = np.concatenate([W1, W1As, W1Ad], axis=1).astype(BF)   # [192, 272]
    W2 = np.asarray(inputs["W2"], np.float32)          # [256, 8]
    W2As = W2 @ np.asarray(inputs["att_src2"], np.float32).reshape(OUT, 1)
    W2Ad = W2 @ np.asarray(inputs["att_dst2"], np.float32).reshape(OUT, 1)
    W2ex = np.concatenate([W2, W2As, W2Ad], axis=1).astype(BF)   # [256, 10]
    Wemb = np.asarray(inputs["W_emb"], np.float32)
    Wemb1 = np.concatenate(
        [Wemb, np.asarray(inputs["b_emb"], np.float32)[None, :]],
        axis=0).astype(BF)                             # [33, 64]
    wparts = {
        "W1t": np.ascontiguousarray(W1ex[:HIGH]),
        "W1b": np.ascontiguousarray(W1ex[HIGH:]),
        "Wemb": Wemb1,
        "W2e": np.ascontiguousarray(
            W2ex.reshape(2, P, 10).transpose(1, 0, 2)),          # [P, 2, 10]
        "idn": np.eye(P, dtype=np.float32).astype(BF),
        "iot": np.broadcast_to(np.arange(P, dtype=np.float32),
                               (P, P)).astype(BF).copy(),
        "b1b": np.broadcast_to(np.asarray(inputs["b1"], np.float32),
                               (P, IN2)).astype(BF).copy(),
        "b2b": np.broadcast_to(np.asarray(inputs["b2"], np.float32),
                               (P, OUT)).astype(BF).copy(),
    }
    for name, arr in wparts.items():
        off, n, shape = secs[name]
        assert arr.shape == shape, (name, arr.shape, shape)
        blob[:, off:off + n // 2] = arr.reshape(-1).view(np.int32)[None, :]

    return blob, C_t, TC, NSUP, tile_of_chunk, first_chunk, Lr


# ---------------------------------------------------------------- device build
def _build(C_t, TC, NSUP, tile_of_chunk, first_chunk, Lr):
    nc = bacc.Bacc("TRN2", target_bir_lowering=False, debug=False,
                   num_devices=NCORES)
    bf, f32, i32 = mybir.dt.bfloat16, mybir.dt.float32, mybir.dt.int32

    f8 = mybir.dt.float8e4
    u16, u8 = mybir.dt.uint16, mybir.dt.uint8
    FEAT = nc.dram_tensor("FEAT", [FROWS, SH], f8, kind="ExternalInput")
    BLOB = nc.dram_tensor("BLOB", [Lr], i32, kind="ExternalInput")
    out_d = nc.dram_tensor("out", [SH, OUT], bf, kind="ExternalOutput")

    adt1 = nc.dram_tensor("adt1", [SH, 16], bf)
    adt2 = nc.dram_tensor("adt2", [SH, 2], bf)
    h1x = nc.dram_tensor("h1x", [N, 272], bf, addr_space="Shared")
    h2xf = nc.dram_tensor("h2xf", [N, 10], bf, addr_space="Shared")

    S1, S2, secs, Lr2 = _blob_layout(NSUP)
    assert Lr2 == Lr

    def sec_ap(name):
        off, n, shape = secs[name]
        ap = BLOB[off:off + n // 2].bitcast(bf)
        if len(shape) == 2:
            return ap.rearrange("(a b) -> a b", b=shape[1])
        return ap.rearrange("(a b c) -> a b c", b=shape[1], c=shape[2])

    def srcg_ap(s):
        return BLOB[s * P * B // 2:(s + 1) * P * B // 2] \
            .bitcast(u16).rearrange("(p b) -> p b", b=B)

    def dl_ap(s):
        off = secs["DL"][0]
        return BLOB[off + s * P * B // 4:off + (s + 1) * P * B // 4] \
            .bitcast(u8).rearrange("(p b) -> p b", b=B)

    def dstt_ap(t):
        base = S1 // 2 + t * P
        return BLOB[base:base + P].rearrange("(p a) -> p a", a=1)

    with tile.TileContext(nc) as tc:
        with tc.tile_pool(name="const", bufs=1) as cpool, \
             tc.tile_pool(name="sb", bufs=3) as sb, \
             tc.tile_pool(name="gat", bufs=3) as gat, \
             tc.tile_pool(name="psA", bufs=2, space="PSUM") as psA, \
             tc.tile_pool(name="psB", bufs=3, space="PSUM") as psB, \
             tc.tile_pool(name="dram", bufs=1, space="DRAM") as dram:

            h1l = dram.tile([SH, 272], bf)

            w1t = cpool.tile([HIGH, 272], bf)
            nc.sync.dma_start(out=w1t[:], in_=sec_ap("W1t"))
            w1b = cpool.tile([EMB, 272], bf)
            nc.sync.dma_start(out=w1b[:], in_=sec_ap("W1b"))
            wem = cpool.tile([LOW + 1, EMB], bf)
            nc.sync.dma_start(out=wem[:], in_=sec_ap("Wemb"))
            w2e = cpool.tile([P, 2, 10], bf)
            nc.sync.dma_start(out=w2e[:], in_=sec_ap("W2e"))
            b1s = cpool.tile([P, IN2], bf)
            nc.sync.dma_start(out=b1s[:], in_=sec_ap("b1b"))
            b2s = cpool.tile([P, OUT], bf)
            nc.sync.dma_start(out=b2s[:], in_=sec_ap("b2b"))
            ids = cpool.tile([P, P], bf)
            nc.sync.dma_start(out=ids[:], in_=sec_ap("idn"))
            iot = cpool.tile([P, P], bf)
            nc.sync.dma_start(out=iot[:], in_=sec_ap("iot"))

            # -------- Phase A: layer-1 tables for this core's SH nodes -------
            for ntile in range(NT):
                n0 = ntile * P
                w = min(P, SH - n0)
                ht8 = sb.tile([P, P], f8, tag="ht8")
                nc.sync.dma_start(out=ht8[:, :w], in_=FEAT[0:HIGH, n0:n0 + w])
                lt8 = sb.tile([LOW + 1, P], f8, tag="lt8")
                nc.sync.dma_start(out=lt8[:, :w], in_=FEAT[HIGH:FROWS, n0:n0 + w])
                ht = sb.tile([P, P], bf, tag="ht")
                nc.scalar.activation(ht[:, :w], ht8[:, :w], AF.Copy)
                lt = sb.tile([LOW + 1, P], bf, tag="lt")
                nc.scalar.activation(lt[:, :w], lt8[:, :w], AF.Copy)
                embp = psB.tile([EMB, P], f32, tag="pB")
                nc.tensor.matmul(out=embp[:, :w], lhsT=wem[:], rhs=lt[:, :w],
                                 start=True, stop=True)
                # elu(v) = max(v,0)-1 + exp(-relu(-v))
                tm = sb.tile([EMB, P], f32, tag="tm")
                nc.scalar.activation(tm[:, :w], embp[:, :w], AF.Relu, scale=-1.0)
                te = sb.tile([EMB, P], f32, tag="te")
                nc.scalar.activation(te[:, :w], tm[:, :w], AF.Exp, scale=-1.0)
                tr = sb.tile([EMB, P], f32, tag="tr")
                nc.vector.tensor_scalar(tr[:, :w], embp[:, :w], 0.0, -1.0,
                                        ALU.max, ALU.add)
                embs = sb.tile([EMB, P], bf, tag="embs")
                nc.vector.tensor_tensor(embs[:, :w], tr[:, :w], te[:, :w],
                                        ALU.add)
                h1p = psA.tile([P, 512], f32, tag="pA")
                nc.tensor.matmul(out=h1p[:w, 0:272], lhsT=ht[:, :w], rhs=w1t[:],
                                 start=True, stop=False)
                nc.tensor.matmul(out=h1p[:w, 0:272], lhsT=embs[:, :w],
                                 rhs=w1b[:], start=False, stop=True)
                h1s = sb.tile([P, 272], bf, tag="h1s")
                nc.scalar.activation(h1s[:w, 0:256], h1p[:w, 0:256], AF.Copy)
                ads = sb.tile([P, 16], bf, tag="ads")
                nc.scalar.activation(h1s[:w, 256:264], h1p[:w, 256:264], AF.Exp)
                nc.scalar.activation(h1s[:w, 264:272], h1p[:w, 256:264], AF.Exp,
                                     scale=NEG)
                nc.scalar.activation(ads[:w, 0:8], h1p[:w, 264:272], AF.Exp)
                nc.scalar.activation(ads[:w, 8:16], h1p[:w, 264:272], AF.Exp,
                                     scale=NEG)
                nc.sync.dma_start(out=h1l[n0:n0 + w, :], in_=h1s[:w])
                nc.sync.dma_start(out=adt1[n0:n0 + w, :], in_=ads[:w])

            # -------- replicate the layer-1 table --------
            nc.gpsimd.collective_compute(
                "AllGather", ALU.bypass,
                replica_groups=[list(range(NCORES))],
                ins=[h1l.opt()], outs=[h1x[:].opt()])

            # ---------------- L1 edge pass ----------------
            h2xl = dram.tile([SH, 10], bf)

            acc_of_tile = {}
            adt_of_tile = {}

            def l1_epilogue(t):
                rows = P if t < NT - 1 else LAST_ROWS
                acc = acc_of_tile.pop(t)
                rz = sb.tile([P, 8], f32, tag="rz")
                nc.vector.reciprocal(rz[:rows], acc[:rows, 256:264])
                xr = sb.tile([P, IN2], f32, tag="xr")
                nc.vector.tensor_tensor(
                    xr[:rows],
                    acc[:rows, 0:256].rearrange("p (h j) -> p h j", j=HID),
                    rz[:rows, :, None].to_broadcast([rows, 8, HID]), ALU.mult)
                nc.vector.tensor_tensor(xr[:rows], xr[:rows], b1s[:rows],
                                        ALU.add)
                tm = sb.tile([P, IN2], f32, tag="etm")
                nc.scalar.activation(tm[:rows], xr[:rows], AF.Relu, scale=-1.0)
                te = sb.tile([P, IN2], f32, tag="ete")
                nc.scalar.activation(te[:rows], tm[:rows], AF.Exp, scale=-1.0)
                tr = sb.tile([P, IN2], f32, tag="etr")
                nc.vector.tensor_scalar(tr[:rows], xr[:rows], 0.0, -1.0,
                                        ALU.max, ALU.add)
                x2 = sb.tile([P, IN2], bf, tag="x2")
                if rows < P:
                    nc.vector.memset(x2[:], 0.0)
                nc.vector.tensor_tensor(x2[:rows], tr[:rows], te[:rows],
                                        ALU.add)
                # x2T blocks + h2x row
                x2tb = sb.tile([P, 2, P], bf, tag="x2tb")
                for k in range(2):
                    tp = psB.tile([P, P], bf, tag="pB")
                    nc.tensor.transpose(out=tp[:], in_=x2[:, k * P:(k + 1) * P],
                                        identity=ids[:])
                    nc.vector.tensor_copy(x2tb[:, k, :], tp[:])
                h2p = psB.tile([P, 16], f32, tag="pB")
                for k in range(2):
                    nc.tensor.matmul(out=h2p[:, 0:10], lhsT=x2tb[:, k, :],
                                     rhs=w2e[:, k, :], start=(k == 0),
                                     stop=(k == 1))
                h2r = sb.tile([P, 10], bf, tag="h2r")
                nc.scalar.activation(h2r[:rows, 0:8], h2p[:rows, 0:8], AF.Copy)
                nc.scalar.activation(h2r[:rows, 8:9], h2p[:rows, 8:9], AF.Exp)
                nc.scalar.activation(h2r[:rows, 9:10], h2p[:rows, 8:9], AF.Exp,
                                     scale=NEG)
                a2r = sb.tile([P, 2], bf, tag="a2r")
                nc.scalar.activation(a2r[:rows, 0:1], h2p[:rows, 9:10], AF.Exp)
                nc.scalar.activation(a2r[:rows, 1:2], h2p[:rows, 9:10], AF.Exp,
                                     scale=NEG)
                nc.sync.dma_start(out=h2xl[t * P:t * P + rows, :], in_=h2r[:rows])
                nc.sync.dma_start(out=adt2[t * P:t * P + rows, :], in_=a2r[:rows])

            for s in range(NSUP):
                c0 = s * B
                it16 = gat.tile([P, B], u16, tag="it16")
                nc.sync.dma_start(out=it16[:], in_=srcg_ap(s))
                it = gat.tile([P, B], i32, tag="it")
                nc.vector.tensor_copy(it[:], it16[:])
                dlt8 = gat.tile([P, B], u8, tag="dlt8")
                nc.sync.dma_start(out=dlt8[:], in_=dl_ap(s))
                dlt = gat.tile([P, B], bf, tag="dlt")
                nc.vector.tensor_copy(dlt[:], dlt8[:])
                ssb = gat.tile([P, B * P], bf, tag="ssb")
                nc.vector.tensor_tensor(
                    ssb[:].rearrange("p (b q) -> p b q", q=P),
                    dlt[:, :, None].to_broadcast([P, B, P]),
                    iot[:, None, :].to_broadcast([P, B, P]), ALU.is_equal)
                sts = gat.tile([P, B * P], bf, tag="sts")
                for ci in range(B):
                    tpp = psB.tile([P, P], bf, tag="pB", name=f"stp{ci}")
                    nc.tensor.transpose(out=tpp[:],
                                        in_=ssb[:, ci * P:(ci + 1) * P],
                                        identity=ids[:])
                    nc.scalar.activation(sts[:, ci * P:(ci + 1) * P], tpp[:],
                                         AF.Copy)
                hg = gat.tile([P, B, 272], bf, tag="hg")
                adp = psB.tile([P, B * 16], f32, tag="pAD")
                for ci in range(B):
                    c = c0 + ci
                    t = int(tile_of_chunk[c])
                    if c == int(first_chunk[t]):
                        dtt = sb.tile([P, 1], i32, tag="dtt")
                        nc.sync.dma_start(out=dtt[:], in_=dstt_ap(t))
                        adtt = sb.tile([P, 16], bf, tag=f"adtt{t % 3}")
                        nc.gpsimd.indirect_dma_start(
                            out=adtt[:], out_offset=None, in_=adt1[:],
                            in_offset=bass.IndirectOffsetOnAxis(
                                ap=dtt[:, :1], axis=0))
                        adt_of_tile[t] = adtt
                        acc_of_tile[t] = psA.tile([P, 512], f32, tag="pA",
                                                  name=f"acc{t}")
                    nc.gpsimd.indirect_dma_start(
                        out=hg[:, ci, :], out_offset=None, in_=h1x[:],
                        in_offset=bass.IndirectOffsetOnAxis(
                            ap=it[:, ci:ci + 1], axis=0))
                    nc.tensor.matmul(out=adp[:, ci * 16:(ci + 1) * 16],
                                     lhsT=sts[:, ci * P:(ci + 1) * P],
                                     rhs=adt_of_tile[t][:], start=True,
                                     stop=True)
                # batched attention weights
                t1 = gat.tile([P, B * 8], f32, tag="t1")
                nc.vector.tensor_tensor(
                    t1[:].rearrange("p (b h) -> p b h", h=8),
                    hg[:, :, 256:264],
                    adp[:].rearrange("p (b h) -> p b h", h=16)[:, :, 0:8],
                    ALU.mult)
                t2 = gat.tile([P, B * 8], f32, tag="t2")
                nc.vector.tensor_tensor(
                    t2[:].rearrange("p (b h) -> p b h", h=8),
                    hg[:, :, 264:272],
                    adp[:].rearrange("p (b h) -> p b h", h=16)[:, :, 8:16],
                    ALU.mult)
                nc.vector.tensor_tensor(
                    hg[:, :, 256:264],
                    t1[:].rearrange("p (b h) -> p b h", h=8),
                    t2[:].rearrange("p (b h) -> p b h", h=8),
                    ALU.max)
                nc.vector.tensor_tensor(
                    hg[:, :, 0:256].rearrange("p b (h j) -> p b h j", j=HID),
                    hg[:, :, 0:256].rearrange("p b (h j) -> p b h j", j=HID),
                    hg[:, :, 256:264][:, :, :, None].to_broadcast(
                        [P, B, 8, HID]),
                    ALU.mult)
                for ci in range(B):
                    c = c0 + ci
                    t = int(tile_of_chunk[c])
                    last = (c == int(first_chunk[t]) + int(C_t[t]) - 1)
                    nc.tensor.matmul(out=acc_of_tile[t][:, 0:264],
                                     lhsT=ssb[:, ci * P:(ci + 1) * P],
                                     rhs=hg[:, ci, 0:264],
                                     start=(c == int(first_chunk[t])),
                                     stop=last)
                    if last:
                        l1_epilogue(t)

            # ---------------- AllGather layer-2 table ----------------
            nc.gpsimd.collective_compute(
                "AllGather", ALU.bypass,
                replica_groups=[list(range(NCORES))],
                ins=[h2xl.opt()], outs=[h2xf[:].opt()])

            # ---------------- L2 edge pass ----------------
            acc2_of_tile = {}
            adt2_of_tile = {}

            def l2_epilogue(t):
                rows = P if t < NT - 1 else LAST_ROWS
                acc = acc2_of_tile.pop(t)
                rz = sb.tile([P, 1], f32, tag="rz2")
                nc.vector.reciprocal(rz[:rows], acc[:rows, 8:9])
                o = sb.tile([P, OUT], f32, tag="o2")
                nc.vector.tensor_tensor(
                    o[:rows], acc[:rows, 0:8],
                    rz[:rows, :].to_broadcast([rows, OUT]), ALU.mult)
                nc.vector.tensor_tensor(o[:rows], o[:rows], b2s[:rows], ALU.add)
                ex = sb.tile([P, OUT], f32, tag="ex2")
                nc.scalar.activation(ex[:rows], o[:rows], AF.Exp)
                sm = sb.tile([P, 1], f32, tag="sm2")
                nc.vector.reduce_sum(sm[:rows], ex[:rows],
                                     axis=mybir.AxisListType.X)
                lg = sb.tile([P, 1], f32, tag="lg2")
                nc.scalar.activation(lg[:rows], sm[:rows], AF.Ln)
                fo = sb.tile([P, OUT], bf, tag="fo2")
                nc.vector.tensor_tensor(
                    fo[:rows], o[:rows],
                    lg[:rows, :].to_broadcast([rows, OUT]), ALU.subtract)
                nc.sync.dma_start(out=out_d[t * P:t * P + rows, :], in_=fo[:rows])

            for s in range(NSUP):
                c0 = s * B
                it16 = gat.tile([P, B], u16, tag="it16")
                nc.sync.dma_start(out=it16[:], in_=srcg_ap(s))
                it = gat.tile([P, B], i32, tag="it")
                nc.vector.tensor_copy(it[:], it16[:])
                dlt8 = gat.tile([P, B], u8, tag="dlt8")
                nc.sync.dma_start(out=dlt8[:], in_=dl_ap(s))
                dlt = gat.tile([P, B], bf, tag="dlt")
                nc.vector.tensor_copy(dlt[:], dlt8[:])
                ssb = gat.tile([P, B * P], bf, tag="ssb")
                nc.vector.tensor_tensor(
                    ssb[:].rearrange("p (b q) -> p b q", q=P),
                    dlt[:, :, None].to_broadcast([P, B, P]),
                    iot[:, None, :].to_broadcast([P, B, P]), ALU.is_equal)
                sts = gat.tile([P, B * P], bf, tag="sts")
                for ci in range(B):
                    tpp = psB.tile([P, P], bf, tag="pB", name=f"stp{ci}")
                    nc.tensor.transpose(out=tpp[:],
                                        in_=ssb[:, ci * P:(ci + 1) * P],
                                        identity=ids[:])
                    nc.scalar.activation(sts[:, ci * P:(ci + 1) * P], tpp[:],
                                         AF.Copy)
                hg2 = gat.tile([P, B, 10], bf, tag="hg2")
                adp2 = psB.tile([P, B * 2], f32, tag="pAD")
                for ci in range(B):
                    c = c0 + ci
                    t = int(tile_of_chunk[c])
                    if c == int(first_chunk[t]):
                        a2t = sb.tile([P, 2], bf, tag=f"a2t{t % 3}")
                        rows = P if t < NT - 1 else LAST_ROWS
                        if rows < P:
                            nc.vector.memset(a2t[:], 0.0)
                        nc.sync.dma_start(out=a2t[:rows],
                                          in_=adt2[t * P:t * P + rows, :])
                        adt2_of_tile[t] = a2t
                        acc2_of_tile[t] = psA.tile([P, 512], f32, tag="pA",
                                                   name=f"acc2_{t}")
                    nc.gpsimd.indirect_dma_start(
                        out=hg2[:, ci, :], out_offset=None, in_=h2xf[:],
                        in_offset=bass.IndirectOffsetOnAxis(
                            ap=it[:, ci:ci + 1], axis=0))
                    nc.tensor.matmul(out=adp2[:, ci * 2:(ci + 1) * 2],
                                     lhsT=sts[:, ci * P:(ci + 1) * P],
                                     rhs=adt2_of_tile[t][:], start=True,
                                     stop=True)
                t1 = gat.tile([P, B], f32, tag="t1b")
                nc.vector.tensor_tensor(
                    t1[:, :, None], hg2[:, :, 8:9],
                    adp2[:].rearrange("p (b k) -> p b k", k=2)[:, :, 0:1],
                    ALU.mult)
                t2 = gat.tile([P, B], f32, tag="t2b")
                nc.vector.tensor_tensor(
                    t2[:, :, None], hg2[:, :, 9:10],
                    adp2[:].rearrange("p (b k) -> p b k", k=2)[:, :, 1:2],
                    ALU.mult)
                nc.vector.tensor_tensor(
                    hg2[:, :, 8:9], t1[:, :, None], t2[:, :, None], ALU.max)
                nc.vector.tensor_tensor(
                    hg2[:, :, 0:8], hg2[:, :, 0:8],
                    hg2[:, :, 8:9].to_broadcast([P, B, OUT]), ALU.mult)
                for ci in range(B):
                    c = c0 + ci
                    t = int(tile_of_chunk[c])
                    last = (c == int(first_chunk[t]) + int(C_t[t]) - 1)
                    nc.tensor.matmul(out=acc2_of_tile[t][:, 0:9],
                                     lhsT=ssb[:, ci * P:(ci + 1) * P],
                                     rhs=hg2[:, ci, 0:9],
                                     start=(c == int(first_chunk[t])),
                                     stop=last)
                    if last:
                        l2_epilogue(t)

    if not nc.is_finalized():
        nc.finalize()
    return nc


# ---------------------------------------------------------------- runner
_CACHE = {}   # structure key -> (nc, runner)


def _make_runner(nc):
    """Cached-jit replica of bass2jax.run_bass_via_pjrt (axon path)."""
    from concourse.bass2jax import (install_neuronx_cc_hook,
                                    partition_id_tensor, _bass_exec_p)
    install_neuronx_cc_hook()
    partition_name = (nc.partition_id_tensor.name
                      if nc.partition_id_tensor else None)
    in_names, out_names, out_avals = [], [], []
    for alloc in nc.m.functions[0].allocations:
        if not isinstance(alloc, mybir.MemoryLocationSet):
            continue
        name = alloc.memorylocations[0].name
        if alloc.kind == "ExternalInput":
            if name != partition_name:
                in_names.append(name)
        elif alloc.kind == "ExternalOutput":
            out_names.append(name)
            out_avals.append(jax.core.ShapedArray(
                tuple(alloc.tensor_shape), mybir.dt.np(alloc.dtype)))
    n_params = len(in_names)
    n_outs = len(out_avals)
    in_names_all = in_names + out_names + (
        [partition_name] if partition_name else [])
    donate = tuple(range(n_params, n_params + n_outs))

    def _body(*args):
        operands = list(args)
        if partition_name is not None:
            operands.append(partition_id_tensor())
        outs = _bass_exec_p.bind(
            *operands, out_avals=tuple(out_avals),
            in_names=tuple(in_names_all), out_names=tuple(out_names),
            lowering_input_output_aliases=(), sim_require_finite=True,
            sim_require_nnan=True, nc=nc)
        return tuple(outs)

    devices = jax.devices()[:NCORES]
    mesh = Mesh(np.asarray(devices), ("core",))
    sharding = NamedSharding(mesh, PartitionSpec("core"))
    in_specs = (PartitionSpec("core"),) * (n_params + n_outs)
    out_specs = (PartitionSpec("core"),) * len(out_names)
    fn = jax.jit(
        shard_map(_body, mesh=mesh, in_specs=in_specs, out_specs=out_specs,
                  check_rep=False),
        donate_argnums=donate, keep_unused=True)
    return fn, in_names, out_names, out_avals, sharding


_PREV_OUT = [None]   # previous call's device output, donated as the next
                     # zeros-input (the kernel writes every output element)


def kernel(**inputs):
    FG = _prep_feat(inputs)
    # fire the (dominant) feature upload before doing edge bucketing so the
    # tunnel transfer overlaps host prep
    dfeat = dzeros = None
    try:
        devices = jax.devices()[:NCORES]
        mesh = Mesh(np.asarray(devices), ("core",))
        sharding = NamedSharding(mesh, PartitionSpec("core"))
        dfeat = jax.device_put(FG, sharding)
        if _PREV_OUT[0] is not None:
            dzeros = _PREV_OUT[0]
            _PREV_OUT[0] = None
        else:
            dzeros = jax.device_put(
                np.zeros((NCORES * SH, OUT), BF), sharding)
    except Exception:
        dfeat = dzeros = None

    blob, C_t, TC, NSUP, toc, fc, Lr = _prep_edges(inputs)
    key = (TC, NSUP, tuple(int(x) for x in C_t))
    if key not in _CACHE:
        nc = _build(C_t, TC, NSUP, toc, fc, Lr)
        _CACHE[key] = (nc, _make_runner(nc))
    nc, (fn, in_names, out_names, out_avals, sharding) = _CACHE[key]

    host_in = {"FEAT": FG, "BLOB": blob.reshape(-1)}
    dev_in = []
    for name in in_names:
        if name == "FEAT" and dfeat is not None:
            dev_in.append(dfeat)
        else:
            dev_in.append(jax.device_put(host_in[name], sharding))
    if dzeros is None:
        dzeros = jax.device_put(
            np.zeros((NCORES * SH, OUT), BF), sharding)
    out_arrs = fn(*dev_in, dzeros)
    dev_out = out_arrs[out_names.index("out")]
    out = np.asarray(dev_out).astype(np.float32)
    _PREV_OUT[0] = dev_out
    return out.reshape(N, OUT)
